# revision 1
# baseline (speedup 1.0000x reference)
"""Trainium2 Bass kernel: Whisper-style self-attention (B=4, S=1500, D=1280, H=20).

Sharding: core c = 2*b + g handles batch b (of 4) and head-group g (of 2,
10 heads each).  Q/K/V projections column-sharded over the head group,
attention sharded by (batch, head), output projection row-sharded; the two
head-group partials of each batch are summed on the host (plus bias terms).

v2 dataflow (all matmul operands fp16 -> 1.0 cycles/row; f32 PSUM):
  xT [1280,1500] fp16 -> qT,kT [128,5,S] fp16 (qT scaled 1/8 + bq),
  v [128,12,10,65] fp16 (64 v cols + ones col per head -> softmax Z).
  Per (head, q-chunk) unit: scoresT = kT.T@qT per k-tile (K=64), Exp on ACT
  -> expT fp16 [128,12,cw];  PV swapped: ctx[q-tile(128 part), 65] accum in
  PSUM over 12 k-tiles (col 64 = Z) -- streams 65 rows/matmul instead of
  ~500, halving PV tensor-engine time vs the [65, q] orientation.
  DVE computes 1/Z; Pool multiplies ctx*(1/Z) -> ctxq fp16 [128,4,640] per
  chunk; DMA XBAR transpose (idle DMA engines) flips it into ctxT
  [128,5,S'] fp16; O-proj fp16 -> out f32 (Pool drains PSUM, DVE-queue DMA).

Scheduling: units are woven as [score-pair, pv-piece, score-pair, ...] with
the pv of unit u-2 riding between unit u's score pairs, so the PE never
waits on ACT exp draining the 2-buf score PSUM.  Projection / O-proj work
sits in a prerequisite-keyed FIFO of ~1-2us pieces; each unit first drains
the pieces its scores/pv depend on, then pops a tunable extra budget.
"""
import sys
sys.path.insert(0, "/opt/trn_rl_repo")

from collections import deque
from contextlib import ExitStack
import numpy as np

import concourse.bass as bass
import concourse.tile as tile
from concourse import bacc, mybir
from concourse.bass_utils import run_bass_kernel_spmd

dt = mybir.dt
AF = mybir.ActivationFunctionType
ALU = mybir.AluOpType

N_CORES = 8
B, S, D = 4, 1500, 1280
H, DH = 20, 64
G = 2
DG = D // G            # 640
HPG = H // G           # 10
KD = D // 128          # 10 contraction planes for D
MD = DG // 128         # 5 dh-planes per group
CW = (512, 512, 476)   # q/proj chunk widths
CO = (0, 512, 1024)
NS = 3
KS = (S + 127) // 128  # 12 k-tiles (11*128 + 92)
SP_ = 12 * 128         # 1536: padded S for ctxT columns
ON = (512, 512, 256)   # o-proj n chunks

_CACHE = {}


def _sk(i):
    return min(128, S - i * 128)


def build():
    nc = bacc.Bacc("TRN2", target_bir_lowering=False, debug=False,
                   num_devices=N_CORES)
    xt_d = nc.dram_tensor("xt", [128, S, KD], dt.float16,
                          kind="ExternalInput").ap()
    wq_d = nc.dram_tensor("wq", [MD, 128, KD, 128], dt.float16,
                          kind="ExternalInput").ap()
    wk_d = nc.dram_tensor("wk", [MD, 128, KD, 128], dt.float16,
                          kind="ExternalInput").ap()
    wv_d = nc.dram_tensor("wv", [G, 128, KD, 320], dt.float16,
                          kind="ExternalInput").ap()
    wo_d = nc.dram_tensor("wo", [DG, D], dt.float16, kind="ExternalInput").ap()
    bq_d = nc.dram_tensor("bq", [128, MD], dt.float32, kind="ExternalInput").ap()
    out_d = nc.dram_tensor("out", [S, D], dt.float32, kind="ExternalOutput").ap()

    wo_r = wo_d.rearrange("(k p) n -> p k n", p=128)

    with tile.TileContext(nc) as tc, ExitStack() as octx:
        persist = octx.enter_context(tc.tile_pool(name="persist", bufs=1))
        wqkp = octx.enter_context(tc.tile_pool(name="wqk", bufs=7))
        wvp = octx.enter_context(tc.tile_pool(name="wv", bufs=2))
        epool = octx.enter_context(tc.tile_pool(name="expT", bufs=4))
        cqpool = octx.enter_context(tc.tile_pool(name="ctxq", bufs=3))
        zpool = octx.enter_context(tc.tile_pool(name="z", bufs=3))
        opool = octx.enter_context(tc.tile_pool(name="ob", bufs=5))
        ps1 = octx.enter_context(tc.tile_pool(name="ps1", bufs=2, space="PSUM"))
        ps2 = octx.enter_context(tc.tile_pool(name="ps2", bufs=2, space="PSUM"))
        psc = octx.enter_context(tc.tile_pool(name="psc", bufs=2, space="PSUM"))

        def pv_psum():
            return psc.tile([128, 4, DH + 1], dt.float32,
                            name="pvps", tag="psc")

        xt_s = persist.tile([128, S, KD], dt.float16, tag="xt")
        qT = persist.tile([128, MD, S], dt.float16, tag="qT")
        kT = persist.tile([128, MD, S], dt.float16, tag="kT")
        v = persist.tile([128, KS, HPG, DH + 1], dt.float16, tag="v")
        ctxT = persist.tile([128, MD, SP_], dt.float16, tag="ctxT")
        wo_s = persist.tile([128, MD, D], dt.float16, tag="wo")
        bq_s = persist.tile([128, MD], dt.float32, tag="bq")

        wts = {}

        def qk_dma(m, which):
            w_d = wq_d if which == "q" else wk_d
            wt = wqkp.tile([128, KD, 128], dt.float16, name=f"w{which}{m}",
                           tag="wqk")
            nc.sync.dma_start(out=wt[:], in_=w_d[m])
            wts[which, m] = wt

        def v_dma(n):
            wt = wvp.tile([128, KD, 320], dt.float16, name=f"wv{n}", tag="wv")
            nc.sync.dma_start(out=wt[:], in_=wv_d[n])
            wts["v", n] = wt

        # startup DMA order is tuned so the PE preamble below never waits:
        # x[0:256] first (smallest useful grain), then plane-0/1 weights,
        # then the rest of x; wv0/bq after.  The preamble's chain order
        # consumes exactly what has landed by the time PE reaches it.
        nc.sync.dma_start(out=xt_s[:, 0:128, :], in_=xt_d[:, 0:128, :])
        qk_dma(0, "k")
        qk_dma(0, "q")
        nc.sync.dma_start(out=bq_s[:], in_=bq_d[:])
        nc.sync.dma_start(out=xt_s[:, 128:512, :], in_=xt_d[:, 128:512, :])
        v_dma(0)
        qk_dma(1, "k")
        qk_dma(1, "q")
        for co in range(512, S, 256):
            cw = min(256, S - co)
            nc.sync.dma_start(out=xt_s[:, co:co + cw, :],
                              in_=xt_d[:, co:co + cw, :])

        from concourse.masks import make_identity
        ident = persist.tile([128, 128], dt.float16, tag="ident")
        make_identity(nc, ident[:])
        ones1 = persist.tile([128, 1], dt.float16, tag="ones1")
        nc.vector.memset(ones1[:], 1.0)
        nc.vector.tensor_copy(v[:, :, :, DH:DH + 1],
                              ones1[:].to_broadcast([128, KS, HPG, 1]))

        def qk_chain(which, m, n, co=None, cw=None):
            """One projection chain: qT/kT plane m, column chunk n. ~2.1us."""
            wt = wts[which, m]
            dst = qT if which == "q" else kT
            if co is None:
                cw, co = CW[n], CO[n]
            ps = ps1.tile([128, 1, 512], dt.float32, tag="ps1")
            for kk in range(KD):
                nc.tensor.matmul(
                    ps[:, 0, 0:cw],
                    lhsT=wt[:, kk, :],
                    rhs=xt_s[:, co:co + cw, kk],
                    start=(kk == 0), stop=(kk == KD - 1))
            osl = dst[:, m, co:co + cw]
            if which == "q":
                nc.vector.tensor_scalar(
                    osl, ps[:, 0, 0:cw], 0.125, bq_s[:, m:m + 1],
                    op0=ALU.mult, op1=ALU.add)
            else:
                nc.vector.tensor_copy(osl, ps[:, 0, 0:cw])

        def v_chain(n, ms):
            """v columns for heads 5n..5n+4, s-tile ms. ~1.3us."""
            wt = wts["v", n]
            sp = _sk(ms)
            ps = ps1.tile([128, 1, 512], dt.float32, tag="ps1")
            for kk in range(KD):
                nc.tensor.matmul(
                    ps[0:sp, 0, 0:320],
                    lhsT=xt_s[:, ms * 128:ms * 128 + sp, kk],
                    rhs=wt[:, kk, :],
                    start=(kk == 0), stop=(kk == KD - 1))
            nc.vector.tensor_copy(
                v[0:sp, ms, n * 5:(n + 1) * 5, 0:DH],
                ps[0:sp, 0, 0:320].rearrange("p (h e) -> p h e", h=5))

        def oproj_chain(qt, ni, act_copy=False, dma_act=False):
            """One o-proj chain: q-tile qt, n-chunk ni. ~1.1us + drain.
            act_copy routes the PSUM drain to the ACT engine (idle at the
            kernel tail) so it can't delay DVE's critical recip/norm."""
            sp = _sk(qt)
            mw = 128 if sp == 128 else 96
            nw = ON[ni]
            noff = CO[ni]
            ps = ps1.tile([128, 1, 512], dt.float32, tag="ps1")
            for kk in range(MD):
                nc.tensor.matmul(
                    ps[0:mw, 0, 0:nw],
                    lhsT=ctxT[:, kk, qt * 128:qt * 128 + mw],
                    rhs=wo_s[:, kk, noff:noff + nw],
                    start=(kk == 0), stop=(kk == MD - 1))
            ob = opool.tile([128, 512], dt.float32, tag="ob")
            if act_copy:
                nc.scalar.copy(ob[0:sp, 0:nw], ps[0:sp, 0, 0:nw])
            else:
                nc.vector.tensor_copy(ob[0:sp, 0:nw], ps[0:sp, 0, 0:nw])
            deng = nc.scalar if dma_act else nc.sync
            deng.dma_start(
                out=out_d[qt * 128:qt * 128 + sp, noff:noff + nw],
                in_=ob[0:sp, 0:nw])

        # ---- prerequisite-keyed filler piece queue --------------------
        pieces = deque()        # (key, fn, cost_ns)
        emitted = set()
        emitted.update([("k", 0, 0), ("k", 0, 1), ("k", 0, 2), ("q", 0, 0)])

        def pop_one():
            key, fn, cost = pieces.popleft()
            fn()
            emitted.add(key)
            return cost

        def drain_until(keys):
            need = [k for k in keys if k not in emitted]
            for k in need:
                while k not in emitted:
                    assert pieces, f"piece schedule missing prerequisite {k}"
                    pop_one()

        def pop_budget(budget):
            while budget > 0 and pieces:
                budget -= pop_one()

        def QK(which, m, n):
            return ((which, m, n), lambda: qk_chain(which, m, n), 2130)

        def VC(n, ms):
            return (("v", n, ms), lambda: v_chain(n, ms), 1330)

        def DMAW(key, fn, *a):
            return (key, lambda: fn(*a), 50)

        def OP(qt, ni):
            return (("op", qt, ni), lambda: oproj_chain(qt, ni), 1070)

        # ---- attention unit, woven ------------------------------------
        pending = deque()       # (h, c, ex, ctxq_tile) awaiting pv
        ctxq_by_c = {}

        def pv_piece(ph, pc_, pex, qt_i, pc_t):
            cw = CW[pc_]
            qco = qt_i * 128
            qw = min(128, cw - qco)
            for kk in range(KS):
                sp = _sk(kk)
                nc.tensor.matmul(
                    pc_t[0:qw, qt_i, :],
                    lhsT=pex[0:sp, kk, qco:qco + qw],
                    rhs=v[0:sp, kk, ph, :],
                    start=(kk == 0), stop=(kk == KS - 1))

        def pv_finish(ph, pc_, pcq, pc_t):
            last = ph == HPG - 1
            if last and pc_ == 2:
                # end-game: per-qtile recip+norm on DVE, then a PE warm
                # chain while norms drain, then PE transposes (53ns each via
                # identity matmul, fp16 PSUM) + DVE copies into ctxT --
                # ~2us lower latency than the DMA XBAR path and keeps the
                # PE p-state hot into the final o-proj.
                for qt_i in range(4):
                    zq = zpool.tile([128, 1, 1], dt.float32, name=f"zq{qt_i}",
                                    tag="zr")
                    nc.vector.reciprocal(zq[:], pc_t[:, qt_i:qt_i + 1,
                                                     DH:DH + 1])
                    nc.vector.tensor_tensor(
                        pcq[:, qt_i, ph * DH:(ph + 1) * DH],
                        pc_t[:, qt_i, 0:DH],
                        zq[:, 0].to_broadcast([128, DH]), op=ALU.mult)
                psts = []
                for qt_i in range(4):
                    pst = psc.tile([128, MD, 128], dt.float16,
                                   name=f"pst{qt_i}", tag="psc")
                    for m in range(MD):
                        nc.tensor.transpose(
                            pst[:, m, :],
                            pcq[0:128, qt_i, m * 128:(m + 1) * 128],
                            ident[:])
                    psts.append(pst)
                for qt_i in range(4):
                    qt = 4 * pc_ + qt_i
                    nc.scalar.copy(
                        ctxT[:, :, qt * 128:qt * 128 + 128], psts[qt_i][:])
                return
            zr = zpool.tile([128, 4, 1], dt.float32, tag="zr")
            nc.vector.reciprocal(zr[:], pc_t[:, :, DH:DH + 1])
            nc.vector.tensor_tensor(
                pcq[:, :, ph * DH:(ph + 1) * DH], pc_t[:, :, 0:DH],
                zr[:].to_broadcast([128, 4, DH]), op=ALU.mult)
            if last:
                for qt_i in range(4):
                    qt = 4 * pc_ + qt_i
                    pp = 128 if _sk(qt) == 128 else 96
                    nc.sync.dma_start_transpose(
                        out=ctxT[:, :, qt * 128:qt * 128 + pp],
                        in_=pcq[0:pp, qt_i, :])

        def get_ctxq(c):
            if c not in ctxq_by_c:
                ctxq_by_c[c] = cqpool.tile([128, 4, DG], dt.float16,
                                           name=f"ctxq{c}", tag="ctxq")
            return ctxq_by_c[c]

        def scores_pair(h, c, kk2, ex):
            base = 64 * (h % 2)
            td = h // 2
            cw, co = CW[c], CO[c]
            ps = ps2.tile([128, 2, 512], dt.float32, tag="ps2")
            for j in range(2):
                kk = kk2 + j
                sp = _sk(kk)
                nc.tensor.matmul(
                    ps[0:sp, j, 0:cw],
                    lhsT=kT[base:base + 64, td, kk * 128:kk * 128 + sp],
                    rhs=qT[base:base + 64, td, co:co + cw],
                    start=True, stop=True)
            nc.scalar.activation(ex[:, kk2:kk2 + 2, 0:cw], ps[:, :, 0:cw],
                                 AF.Exp)

        slot_no = [0]

        def unit(h, c, budget=2200):
            get_ctxq(c)
            m = h // 2
            prereq = [("k", m, 0), ("k", m, 1), ("k", m, 2), ("q", m, c)]
            lag = 3 if slot_no[0] < 5 else 2
            slot_no[0] += 1
            do_pv = len(pending) >= lag
            if do_pv:
                ph = pending[0][0]
                prereq += [("v", ph // 5, ms) for ms in range(KS)]
            drain_until(prereq)
            popped = None
            if do_pv:
                ph, pc_, pex, pcq = pending.popleft()
                popped = (ph, pc_)
                pc_t = pv_psum()
            ex = epool.tile([128, KS, 512], dt.float16, tag="expT")
            for kk2 in range(0, KS, 2):
                scores_pair(h, c, kk2, ex)
                if do_pv and kk2 >= 4:      # weave pv qtiles btwn pairs 3..6
                    pv_piece(ph, pc_, pex, kk2 // 2 - 2, pc_t)
            if do_pv:
                pv_finish(ph, pc_, pcq, pc_t)
            pending.append((h, c, ex, ctxq_by_c[c]))
            pop_budget(budget)
            return popped

        def flush():
            ph, pc_, pex, pcq = pending.popleft()
            drain_until([("v", ph // 5, ms) for ms in range(KS)])
            pc_t = pv_psum()
            for qt_i in range(4):
                if pieces:
                    pop_one()
                pv_piece(ph, pc_, pex, qt_i, pc_t)
            pv_finish(ph, pc_, pcq, pc_t)

        # ---- preamble: unit(0,0)'s score pairs hand-woven between the
        # projection chains they depend on, so ACT exp starts ~8us in.
        qk_chain("k", 0, 0, co=0, cw=128)
        qk_chain("q", 0, 0, co=0, cw=128)
        qk_chain("k", 0, 0, co=128, cw=384)
        qk_chain("q", 0, 0, co=128, cw=384)
        ex0 = epool.tile([128, KS, 512], dt.float16, tag="expT")
        scores_pair(0, 0, 0, ex0)
        scores_pair(0, 0, 2, ex0)
        v_chain(0, 0)
        v_chain(0, 1)
        qk_chain("k", 1, 0)
        qk_chain("q", 1, 0)
        qk_chain("k", 0, 1)
        scores_pair(0, 0, 4, ex0)
        scores_pair(0, 0, 6, ex0)
        v_chain(0, 2)
        v_chain(0, 3)
        qk_chain("k", 1, 1)
        qk_chain("k", 0, 2)
        scores_pair(0, 0, 8, ex0)
        scores_pair(0, 0, 10, ex0)
        pending.append((0, 0, ex0, get_ctxq(0)))
        emitted.update([("k", 1, 0), ("k", 1, 1), ("q", 1, 0)] +
                       [("v", 0, i) for i in range(4)])

        # piece FIFO in first-use order (see unit sequence below).
        pieces.extend([QK("q", 0, 1), QK("q", 1, 1), QK("k", 1, 2),
                       QK("q", 0, 2), QK("q", 1, 2)])
        pieces.extend(VC(0, ms) for ms in range(4, KS))
        pieces.extend([DMAW(("dma", "k2"), qk_dma, 2, "k"),
                       DMAW(("dma", "q2"), qk_dma, 2, "q"),
                       DMAW(("dma", "v1"), v_dma, 1),
                       QK("k", 2, 0), QK("k", 2, 1), QK("k", 2, 2),
                       QK("q", 2, 0), QK("q", 2, 1)])
        pieces.extend(VC(1, ms) for ms in range(KS))
        pieces.extend([DMAW(("dma", "k3"), qk_dma, 3, "k"),
                       DMAW(("dma", "q3"), qk_dma, 3, "q"),
                       QK("k", 3, 0), QK("k", 3, 1), QK("k", 3, 2),
                       QK("q", 3, 0), QK("q", 3, 1),
                       DMAW(("dma", "k4"), qk_dma, 4, "k"),
                       DMAW(("dma", "q4"), qk_dma, 4, "q"),
                       QK("k", 4, 0), QK("k", 4, 1), QK("k", 4, 2),
                       QK("q", 4, 0), QK("q", 4, 1),
                       DMAW(("dma", "wo"),
                            lambda: nc.sync.dma_start(out=wo_s[:],
                                                      in_=wo_r[:]))])
        pieces.extend([QK("q", m, 2) for m in range(2, MD)])

        # unit sequence: chunk-0/1 units interleaved, ordered so units on
        # already-projected kT planes run first -- ACT exp saturates early
        # while the remaining k-plane/v projections drain behind it.
        seq = [(1, 0), (0, 1), (1, 1), (0, 2), (1, 2)]
        for m in range(1, MD - 1):
            seq += [(2 * m, 0), (2 * m, 1), (2 * m + 1, 0), (2 * m + 1, 1)]
        seq += [(8, 0), (9, 0), (8, 1), (9, 1)]
        seq += [(h, 2) for h in range(2, HPG)]
        budgets = {0: 2400, 1: 2400, 2: 2000}
        lean = [True] * 8

        for h, c in seq:
            b = budgets[c]
            if lean:
                b = 1000
                lean.pop()
            popped = unit(h, c, budget=b)
            if popped == (HPG - 1, 0):
                for qt in range(0, 4):
                    pieces.extend(OP(qt, ni) for ni in range(3))
            elif popped == (HPG - 1, 1):
                for qt in range(4, 8):
                    pieces.extend(OP(qt, ni) for ni in range(3))

        flush()                 # pv(7,2)
        for ni in range(3):     # reserved: rides out exp(8,2)/(9,2) latency
            oproj_chain(6, ni)
        flush()                 # pv(8,2)
        for ni in range(3):
            oproj_chain(7, ni)
        # last flush, fully pipelined per qtile: pv(qt) -> recip/norm (DVE)
        # -> PE transpose of qt-1 between pv pieces -> ACT copy, so the
        # final o-proj's first dependencies land while pv still runs.
        lh, lc, lex, lcq = pending.popleft()
        lpc = pv_psum()
        lpsts = []
        for qt_i in range(4):
            pv_piece(lh, lc, lex, qt_i, lpc)
            zq = zpool.tile([128, 1, 1], dt.float32, name=f"lzq{qt_i}",
                            tag="zr")
            nc.vector.reciprocal(zq[:], lpc[:, qt_i:qt_i + 1, DH:DH + 1])
            nc.vector.tensor_tensor(
                lcq[:, qt_i, lh * DH:(lh + 1) * DH],
                lpc[:, qt_i, 0:DH],
                zq[:, 0].to_broadcast([128, DH]), op=ALU.mult)
            if qt_i >= 1:
                pst = psc.tile([128, MD, 128], dt.float16,
                               name=f"lpst{qt_i - 1}", tag="psc")
                for m in range(MD):
                    nc.tensor.transpose(
                        pst[:, m, :],
                        lcq[0:128, qt_i - 1, m * 128:(m + 1) * 128],
                        ident[:])
                lpsts.append(pst)
                nc.scalar.copy(
                    ctxT[:, :, (8 + qt_i - 1) * 128:(8 + qt_i) * 128],
                    pst[:])
        pst = psc.tile([128, MD, 128], dt.float16, name="lpst3", tag="psc")
        for m in range(MD):
            nc.tensor.transpose(
                pst[:, m, :], lcq[0:128, 3, m * 128:(m + 1) * 128], ident[:])
        nc.scalar.copy(ctxT[:, :, 11 * 128:12 * 128], pst[:])
        pop_budget(10**9)       # anything left
        for ni in range(3):
            for qt in range(8, 12):
                oproj_chain(qt, ni, act_copy=(qt == 11 and ni == 2),
                            dma_act=(qt % 2 == 1 and ni < 2))

    nc.compile()
    return nc


def _get_nc():
    if "nc" not in _CACHE:
        _CACHE["nc"] = build()
    return _CACHE["nc"]


def _prep_in_maps(x, Wq, bq, Wk, Wv, Wo):
    in_maps = []
    for c in range(N_CORES):
        b, g = divmod(c, G)
        gs = slice(g * DG, (g + 1) * DG)
        wqT = np.ascontiguousarray(Wq[gs, :].T).astype(np.float16)  # [D, DG]
        wkT = np.ascontiguousarray(Wk[gs, :].T).astype(np.float16)
        wvT = np.ascontiguousarray(Wv[gs, :].T).astype(np.float16)
        # pre-tile: [D, DG] -> [m/n-block, 128 partitions, KD, block]
        wq_h = wqT.reshape(KD, 128, MD, 128).transpose(2, 1, 0, 3)
        wk_h = wkT.reshape(KD, 128, MD, 128).transpose(2, 1, 0, 3)
        wv_h = wvT.reshape(KD, 128, G, 320).transpose(2, 1, 0, 3)
        in_maps.append({
            "xt": np.ascontiguousarray(
                x[b].T.astype(np.float16).reshape(KD, 128, S)
                .transpose(1, 2, 0)),
            "wq": np.ascontiguousarray(wq_h),
            "wk": np.ascontiguousarray(wk_h),
            "wv": np.ascontiguousarray(wv_h),
            "wo": np.ascontiguousarray(Wo[:, gs].T).astype(np.float16),
            "bq": np.ascontiguousarray(
                (0.125 * bq[gs]).astype(np.float32).reshape(MD, 128).T),
        })
    return in_maps


def run(x, Wq, bq, Wk, Wv, bv, Wo, bo, trace=False, **trace_kw):
    x = np.asarray(x, dtype=np.float32)
    Wq = np.asarray(Wq, dtype=np.float32)
    bq = np.asarray(bq, dtype=np.float32)
    Wk = np.asarray(Wk, dtype=np.float32)
    Wv = np.asarray(Wv, dtype=np.float32)
    bv = np.asarray(bv, dtype=np.float32)
    Wo = np.asarray(Wo, dtype=np.float32)
    bo = np.asarray(bo, dtype=np.float32)

    nc = _get_nc()
    in_maps = _prep_in_maps(x, Wq, bq, Wk, Wv, Wo)
    res = None
    for attempt in range(3):
        try:
            res = run_bass_kernel_spmd(nc, in_maps, list(range(N_CORES)),
                                       trace=trace, **trace_kw)
            break
        except Exception:
            # Sporadic NRT_EXEC_UNIT_UNRECOVERABLE on first exec; devices
            # come back after ~75s. Reset the backend and retry.
            if attempt == 2:
                raise
            import time as _time
            import jax as _jax
            _time.sleep(80)
            try:
                _jax.clear_backends()
            except Exception:
                pass
    const = (bv @ Wo.T + bo).astype(np.float32)  # [D]
    out = np.empty((B, S, D), dtype=np.float32)
    for b in range(B):
        out[b] = res.results[2 * b]["out"] + res.results[2 * b + 1]["out"] + const
    return out, res


def kernel(**inputs):
    out, _ = run(**inputs)
    return out



# revision 7
# speedup vs baseline: 1.0557x; 1.0557x over previous
"""Trainium2 Bass kernel: Whisper-style self-attention (B=4, S=1500, D=1280, H=20).

Sharding: core c = 2*b + g handles batch b (of 4) and head-group g (of 2,
10 heads each).  Q/K/V projections column-sharded over the head group,
attention sharded by (batch, head), output projection row-sharded; the two
head-group partials of each batch are summed on the host (plus bias terms).

v3 dataflow: projections run as fp8e4 DoubleRow matmuls (0.5 cycles/row,
2 contraction planes per instruction) with 3-term error compensation:
x ~ x8+dx, W ~ W8+dW (all e4m3, W pre-scaled x32 so residuals clear the
subnormal floor), computing x8W8 + x8dW + dxW8 -- ~fp16 accuracy at 0.75x
the fp16 PE cost for q/k/v and 0.8x for the o-projection (ctx split into
c8+dc on the gpsimd engine after the f16 transpose).  Scale bookkeeping:
q drain mult 0.125/32 (+0.125bq), k/v drains mult 1/32, PV "ones" column
1/16 (so ctxq = 16*ctx, putting the fp8 ctx split in range), o-proj drain
mult 1/512.  Scores/softmax/PV stay fp16 (fp8 attention weights flush to
zero below e4m3's subnormal floor and crater accuracy).

Scheduling: units are woven as [score-pair, pv-piece, score-pair, ...] with
the pv of unit u-2 riding between unit u's score pairs, so the PE never
waits on ACT exp draining the 2-buf score PSUM.  Projection / O-proj work
sits in a prerequisite-keyed FIFO of ~1-2us pieces; each unit first drains
the pieces its scores/pv depend on, then pops a tunable extra budget.
"""
import sys
sys.path.insert(0, "/opt/trn_rl_repo")

from collections import deque
from contextlib import ExitStack
import numpy as np
import ml_dtypes

import concourse.bass as bass
import concourse.tile as tile
from concourse import bacc, mybir
from concourse.bass_utils import run_bass_kernel_spmd

dt = mybir.dt
AF = mybir.ActivationFunctionType
ALU = mybir.AluOpType
PM = mybir.MatmulPerfMode
E4 = ml_dtypes.float8_e4m3

N_CORES = 8
B, S, D = 4, 1500, 1280
H, DH = 20, 64
G = 2
DG = D // G            # 640
HPG = H // G           # 10
KD = D // 128          # 10 contraction planes for D
KP = KD // 2           # 5 DoubleRow plane-pairs
MD = DG // 128         # 5 dh-planes per group
CW = (512, 512, 476)   # q/proj chunk widths
CO = (0, 512, 1024)
NS = 3
KS = (S + 127) // 128  # 12 k-tiles (11*128 + 92)
SP_ = 12 * 128         # 1536: padded S for ctxT columns
ON = (512, 512, 256)   # o-proj n chunks
WSCALE = 32.0          # fp8 weight pre-scale (subnormal headroom)
CSCALE = 16.0          # ctx pre-scale for fp8 split

_CACHE = {}


def _sk(i):
    return min(128, S - i * 128)


def build():
    nc = bacc.Bacc("TRN2", target_bir_lowering=False, debug=False,
                   num_devices=N_CORES)
    x8_d = nc.dram_tensor("x8", [128, KP, 2, SP_], dt.float8e4,
                          kind="ExternalInput").ap()
    dx_d = nc.dram_tensor("dx", [128, KP, 2, SP_], dt.float8e4,
                          kind="ExternalInput").ap()
    # [m, part, which(W8/dW), pair, parity, col]
    wq_d = nc.dram_tensor("wq", [MD, 128, 2, KP, 2, 128], dt.float8e4,
                          kind="ExternalInput").ap()
    wk_d = nc.dram_tensor("wk", [MD, 128, 2, KP, 2, 128], dt.float8e4,
                          kind="ExternalInput").ap()
    wv_d = nc.dram_tensor("wv", [G, 128, 2, KP, 2, 320], dt.float8e4,
                          kind="ExternalInput").ap()
    # o-proj rhs blocks: A=(W8p0,W8p1) B=(W8p2,W8p3) C=(dWp0,dWp1)
    # D=(dWp2,dWp3) E=(W8p4,W8p4) F=(dWp4,dWp4)
    wo_d = nc.dram_tensor("wo", [128, 6, 2, D], dt.float8e4,
                          kind="ExternalInput").ap()
    bq_d = nc.dram_tensor("bq", [128, MD], dt.float32, kind="ExternalInput").ap()
    out_d = nc.dram_tensor("out", [S, D], dt.float32, kind="ExternalOutput").ap()

    with tile.TileContext(nc) as tc, ExitStack() as octx:
        persist = octx.enter_context(tc.tile_pool(name="persist", bufs=1))
        wqkp = octx.enter_context(tc.tile_pool(name="wqk", bufs=6))
        wvp = octx.enter_context(tc.tile_pool(name="wv", bufs=2))
        epool = octx.enter_context(tc.tile_pool(name="expT", bufs=4))
        cqpool = octx.enter_context(tc.tile_pool(name="ctxq", bufs=3))
        zpool = octx.enter_context(tc.tile_pool(name="z", bufs=3))
        opool = octx.enter_context(tc.tile_pool(name="ob", bufs=3))
        ctpool = octx.enter_context(tc.tile_pool(name="ctxT", bufs=4))
        ps1 = octx.enter_context(tc.tile_pool(name="ps1", bufs=2, space="PSUM"))
        ps2 = octx.enter_context(tc.tile_pool(name="ps2", bufs=2, space="PSUM"))
        psc = octx.enter_context(tc.tile_pool(name="psc", bufs=2, space="PSUM"))

        def pv_psum():
            return psc.tile([128, 4, DH + 1], dt.float32,
                            name="pvps", tag="psc")

        x8_s = persist.tile([128, KP, 2, SP_], dt.float8e4, tag="x8")
        dx_s = persist.tile([128, KP, 2, SP_], dt.float8e4, tag="dx")
        qT = persist.tile([128, MD, S], dt.float16, tag="qT")
        kT = persist.tile([128, MD, S], dt.float16, tag="kT")
        v = persist.tile([128, KS, HPG, DH + 1], dt.float16, tag="v")
        cT2 = persist.tile([128, MD, 2, SP_], dt.float8e4, tag="cT2")
        wo_s = persist.tile([128, 6, 2, D], dt.float8e4, tag="wo")
        bq_s = persist.tile([128, MD], dt.float32, tag="bq")

        wts = {}

        def qk_dma(m, which):
            w_d = wq_d if which == "q" else wk_d
            wt = wqkp.tile([128, 2, KP, 2, 128], dt.float8e4,
                           name=f"w{which}{m}", tag="wqk")
            nc.sync.dma_start(out=wt[:], in_=w_d[m])
            wts[which, m] = wt

        def v_dma(n):
            wt = wvp.tile([128, 2, KP, 2, 320], dt.float8e4, name=f"wv{n}",
                          tag="wv")
            nc.sync.dma_start(out=wt[:], in_=wv_d[n])
            wts["v", n] = wt

        # startup DMA order is tuned so the PE preamble below never waits.
        nc.sync.dma_start(out=x8_s[:, :, :, 0:128], in_=x8_d[:, :, :, 0:128])
        nc.sync.dma_start(out=dx_s[:, :, :, 0:128], in_=dx_d[:, :, :, 0:128])
        qk_dma(0, "k")
        qk_dma(0, "q")
        nc.sync.dma_start(out=bq_s[:], in_=bq_d[:])
        nc.sync.dma_start(out=x8_s[:, :, :, 128:512], in_=x8_d[:, :, :, 128:512])
        nc.sync.dma_start(out=dx_s[:, :, :, 128:512], in_=dx_d[:, :, :, 128:512])
        v_dma(0)
        qk_dma(1, "k")
        qk_dma(1, "q")
        for co in range(512, S, 512):
            cw = min(512, S - co)
            nc.sync.dma_start(out=x8_s[:, :, :, co:co + cw],
                              in_=x8_d[:, :, :, co:co + cw])
            nc.sync.dma_start(out=dx_s[:, :, :, co:co + cw],
                              in_=dx_d[:, :, :, co:co + cw])

        from concourse.masks import make_identity
        ident = persist.tile([128, 128], dt.float16, tag="ident")
        make_identity(nc, ident[:])
        ones1 = persist.tile([128, 1], dt.float16, tag="ones1")
        nc.vector.memset(ones1[:], 1.0 / CSCALE)
        nc.vector.tensor_copy(v[:, :, :, DH:DH + 1],
                              ones1[:].to_broadcast([128, KS, HPG, 1]))

        # DoubleRow 3-term order: (x-src, w-sel) per plane-pair
        TERMS = ((0, 0), (1, 0), (0, 1))   # x8*W8, dx*W8, x8*dW

        def qk_chain(which, m, n, co=None, cw=None):
            """One projection chain: qT/kT plane m, column chunk n. ~1.6us."""
            wt = wts[which, m]
            dst = qT if which == "q" else kT
            if co is None:
                cw, co = CW[n], CO[n]
            ps = ps1.tile([128, 1, 512], dt.float32, tag="ps1")
            n_inst = len(TERMS) * KP
            i = 0
            for xsel, wsel in TERMS:
                xsrc = x8_s if xsel == 0 else dx_s
                for pp in range(KP):
                    nc.tensor.matmul(
                        ps[:, 0, 0:cw],
                        lhsT=wt[:, wsel, pp],
                        rhs=xsrc[:, pp, :, co:co + cw],
                        perf_mode=PM.DoubleRow,
                        start=(i == 0), stop=(i == n_inst - 1))
                    i += 1
            osl = dst[:, m, co:co + cw]
            if which == "q":
                nc.vector.tensor_scalar(
                    osl, ps[:, 0, 0:cw], 0.125 / WSCALE, bq_s[:, m:m + 1],
                    op0=ALU.mult, op1=ALU.add)
            else:
                nc.vector.tensor_scalar_mul(osl, ps[:, 0, 0:cw], 1.0 / WSCALE)

        def v_chain(n, ms):
            """v columns for heads 5n..5n+4, s-tile ms. ~1.0us."""
            wt = wts["v", n]
            sp = _sk(ms)
            ps = ps1.tile([128, 1, 512], dt.float32, tag="ps1")
            n_inst = len(TERMS) * KP
            i = 0
            for xsel, wsel in TERMS:
                xsrc = x8_s if xsel == 0 else dx_s
                for pp in range(KP):
                    nc.tensor.matmul(
                        ps[0:sp, 0, 0:320],
                        lhsT=xsrc[:, pp, :, ms * 128:ms * 128 + sp],
                        rhs=wt[:, wsel, pp],
                        perf_mode=PM.DoubleRow,
                        start=(i == 0), stop=(i == n_inst - 1))
                    i += 1
            nc.vector.tensor_scalar_mul(
                v[0:sp, ms, n * 5:(n + 1) * 5, 0:DH],
                ps[0:sp, 0, 0:320].rearrange("p (h e) -> p h e", h=5),
                1.0 / WSCALE)

        def ct_cast(qt, fast=False):
            """ctxT f16 [128, MD, 128] for q-tile qt -> cT2 fp8 (c8, dc).
            fast=True splits the two passes across DVE+Pool (kernel tail)."""
            ct = ct_by_qt.pop(qt)
            qo = qt * 128
            eng = nc.vector if fast else nc.gpsimd
            eng.tensor_copy(cT2[:, :, 0, qo:qo + 128], ct[:])
            nc.gpsimd.tensor_tensor(
                cT2[:, :, 1, qo:qo + 128], ct[:],
                cT2[:, :, 0, qo:qo + 128], op=ALU.subtract)

        def oproj_chain(qt, ni, act_copy=False, dma_act=False):
            """One o-proj chain: q-tile qt, n-chunk ni (8 DoubleRow insts).
            act_copy routes the PSUM drain to the ACT engine (idle at the
            kernel tail) so it can't delay DVE's critical recip/norm."""
            sp = _sk(qt)
            mw = 128 if sp == 128 else 96
            nw = ON[ni]
            noff = CO[ni]
            qo = qt * 128
            ps = ps1.tile([128, 1, 512], dt.float32, tag="ps1")
            insts = [
                (cT2[:, 0:2, 0, qo:qo + mw], 0),
                (cT2[:, 2:4, 0, qo:qo + mw], 1),
                (cT2[:, 0:2, 1, qo:qo + mw], 0),
                (cT2[:, 2:4, 1, qo:qo + mw], 1),
                (cT2[:, 0:2, 0, qo:qo + mw], 2),
                (cT2[:, 2:4, 0, qo:qo + mw], 3),
                (cT2[:, 4, 0:2, qo:qo + mw], 4),
                (cT2[:, 4, 0:2, qo:qo + mw], 5),
            ]
            for i, (lh, wb) in enumerate(insts):
                nc.tensor.matmul(
                    ps[0:mw, 0, 0:nw],
                    lhsT=lh,
                    rhs=wo_s[:, wb, :, noff:noff + nw],
                    perf_mode=PM.DoubleRow,
                    start=(i == 0), stop=(i == len(insts) - 1))
            ob = opool.tile([128, 512], dt.float32, tag="ob")
            if act_copy:
                nc.scalar.mul(ob[0:sp, 0:nw], ps[0:sp, 0, 0:nw],
                              1.0 / (WSCALE * CSCALE))
            else:
                nc.vector.tensor_scalar_mul(ob[0:sp, 0:nw], ps[0:sp, 0, 0:nw],
                                            1.0 / (WSCALE * CSCALE))
            deng = nc.scalar if dma_act else nc.sync
            deng.dma_start(
                out=out_d[qt * 128:qt * 128 + sp, noff:noff + nw],
                in_=ob[0:sp, 0:nw])

        # ---- prerequisite-keyed filler piece queue --------------------
        pieces = deque()        # (key, fn, cost_ns)
        emitted = set()
        emitted.update([("k", 0, 0), ("k", 0, 1), ("k", 0, 2), ("q", 0, 0)])

        def pop_one():
            key, fn, cost = pieces.popleft()
            fn()
            emitted.add(key)
            return cost

        def drain_until(keys):
            need = [k for k in keys if k not in emitted]
            for k in need:
                while k not in emitted:
                    assert pieces, f"piece schedule missing prerequisite {k}"
                    pop_one()

        def pop_budget(budget):
            while budget > 0 and pieces:
                budget -= pop_one()

        def QK(which, m, n):
            return ((which, m, n), lambda: qk_chain(which, m, n), 1650)

        def VC(n, ms):
            return (("v", n, ms), lambda: v_chain(n, ms), 1050)

        def DMAW(key, fn, *a):
            return (key, lambda: fn(*a), 50)

        def OP(qt, ni):
            return (("op", qt, ni), lambda: oproj_chain(qt, ni), 900)

        def CC(qt):
            return (("cc", qt), lambda: ct_cast(qt), 100)

        # ---- attention unit, woven ------------------------------------
        pending = deque()       # (h, c, ex, ctxq_tile) awaiting pv
        ctxq_by_c = {}
        ct_by_qt = {}

        def pv_piece(ph, pc_, pex, qt_i, pc_t):
            cw = CW[pc_]
            qco = qt_i * 128
            qw = min(128, cw - qco)
            for kk in range(KS):
                sp = _sk(kk)
                nc.tensor.matmul(
                    pc_t[0:qw, qt_i, :],
                    lhsT=pex[0:sp, kk, qco:qco + qw],
                    rhs=v[0:sp, kk, ph, :],
                    start=(kk == 0), stop=(kk == KS - 1))

        def new_ct(qt):
            ct = ctpool.tile([128, MD, 128], dt.float16, name=f"ct{qt}",
                             tag="ctxT")
            ct_by_qt[qt] = ct
            return ct

        def pv_finish(ph, pc_, pcq, pc_t):
            last = ph == HPG - 1
            if last and pc_ == 2:
                # end-game: per-qtile recip+norm on DVE, then PE transposes
                # (fp16 PSUM) + ACT copies into per-qtile ctxT tiles.
                for qt_i in range(4):
                    zq = zpool.tile([128, 1, 1], dt.float32, name=f"zq{qt_i}",
                                    tag="zr")
                    nc.vector.reciprocal(zq[:], pc_t[:, qt_i:qt_i + 1,
                                                     DH:DH + 1])
                    nc.vector.tensor_tensor(
                        pcq[:, qt_i, ph * DH:(ph + 1) * DH],
                        pc_t[:, qt_i, 0:DH],
                        zq[:, 0].to_broadcast([128, DH]), op=ALU.mult)
                psts = []
                for qt_i in range(4):
                    pst = psc.tile([128, MD, 128], dt.float16,
                                   name=f"pst{qt_i}", tag="psc")
                    for m in range(MD):
                        nc.tensor.transpose(
                            pst[:, m, :],
                            pcq[0:128, qt_i, m * 128:(m + 1) * 128],
                            ident[:])
                    psts.append(pst)
                for qt_i in range(4):
                    qt = 4 * pc_ + qt_i
                    ct = new_ct(qt)
                    nc.scalar.copy(ct[:], psts[qt_i][:])
                    ct_cast(qt, fast=True)
                return
            zr = zpool.tile([128, 4, 1], dt.float32, tag="zr")
            nc.vector.reciprocal(zr[:], pc_t[:, :, DH:DH + 1])
            nc.vector.tensor_tensor(
                pcq[:, :, ph * DH:(ph + 1) * DH], pc_t[:, :, 0:DH],
                zr[:].to_broadcast([128, 4, DH]), op=ALU.mult)
            if last:
                for qt_i in range(4):
                    qt = 4 * pc_ + qt_i
                    pp = 128 if _sk(qt) == 128 else 96
                    ct = new_ct(qt)
                    nc.sync.dma_start_transpose(
                        out=ct[:, :, 0:pp],
                        in_=pcq[0:pp, qt_i, :])
                    pieces.appendleft(CC(qt))

        def get_ctxq(c):
            if c not in ctxq_by_c:
                ctxq_by_c[c] = cqpool.tile([128, 4, DG], dt.float16,
                                           name=f"ctxq{c}", tag="ctxq")
            return ctxq_by_c[c]

        def scores_pair(h, c, kk2, ex):
            base = 64 * (h % 2)
            td = h // 2
            cw, co = CW[c], CO[c]
            ps = ps2.tile([128, 2, 512], dt.float32, tag="ps2")
            for j in range(2):
                kk = kk2 + j
                sp = _sk(kk)
                nc.tensor.matmul(
                    ps[0:sp, j, 0:cw],
                    lhsT=kT[base:base + 64, td, kk * 128:kk * 128 + sp],
                    rhs=qT[base:base + 64, td, co:co + cw],
                    start=True, stop=True)
            nc.scalar.activation(ex[:, kk2:kk2 + 2, 0:cw], ps[:, :, 0:cw],
                                 AF.Exp)

        slot_no = [0]

        def unit(h, c, budget=2200):
            get_ctxq(c)
            m = h // 2
            prereq = [("k", m, 0), ("k", m, 1), ("k", m, 2), ("q", m, c)]
            lag = 3 if slot_no[0] < 5 else 2
            slot_no[0] += 1
            do_pv = len(pending) >= lag
            if do_pv:
                ph = pending[0][0]
                prereq += [("v", ph // 5, ms) for ms in range(KS)]
            drain_until(prereq)
            popped = None
            if do_pv:
                ph, pc_, pex, pcq = pending.popleft()
                popped = (ph, pc_)
                pc_t = pv_psum()
            ex = epool.tile([128, KS, 512], dt.float16, tag="expT")
            for kk2 in range(0, KS, 2):
                scores_pair(h, c, kk2, ex)
                if do_pv and kk2 >= 4:      # weave pv qtiles btwn pairs 3..6
                    pv_piece(ph, pc_, pex, kk2 // 2 - 2, pc_t)
            if do_pv:
                pv_finish(ph, pc_, pcq, pc_t)
            pending.append((h, c, ex, ctxq_by_c[c]))
            pop_budget(budget)
            return popped

        def flush():
            ph, pc_, pex, pcq = pending.popleft()
            drain_until([("v", ph // 5, ms) for ms in range(KS)])
            pc_t = pv_psum()
            for qt_i in range(4):
                if pieces:
                    pop_one()
                pv_piece(ph, pc_, pex, qt_i, pc_t)
            pv_finish(ph, pc_, pcq, pc_t)

        # ---- preamble: unit(0,0)'s score pairs hand-woven between the
        # projection chains they depend on, so ACT exp starts early.
        qk_chain("k", 0, 0, co=0, cw=128)
        qk_chain("q", 0, 0, co=0, cw=128)
        qk_chain("k", 0, 0, co=128, cw=384)
        qk_chain("q", 0, 0, co=128, cw=384)
        ex0 = epool.tile([128, KS, 512], dt.float16, tag="expT")
        scores_pair(0, 0, 0, ex0)
        scores_pair(0, 0, 2, ex0)
        v_chain(0, 0)
        v_chain(0, 1)
        qk_chain("k", 1, 0)
        qk_chain("q", 1, 0)
        qk_chain("k", 0, 1)
        scores_pair(0, 0, 4, ex0)
        scores_pair(0, 0, 6, ex0)
        v_chain(0, 2)
        v_chain(0, 3)
        qk_chain("k", 1, 1)
        qk_chain("k", 0, 2)
        scores_pair(0, 0, 8, ex0)
        scores_pair(0, 0, 10, ex0)
        pending.append((0, 0, ex0, get_ctxq(0)))
        emitted.update([("k", 1, 0), ("k", 1, 1), ("q", 1, 0)] +
                       [("v", 0, i) for i in range(4)])

        # piece FIFO in first-use order (see unit sequence below).
        pieces.extend([QK("q", 0, 1), QK("q", 1, 1), QK("k", 1, 2),
                       QK("q", 0, 2), QK("q", 1, 2)])
        pieces.extend(VC(0, ms) for ms in range(4, KS))
        pieces.extend([DMAW(("dma", "k2"), qk_dma, 2, "k"),
                       DMAW(("dma", "q2"), qk_dma, 2, "q"),
                       DMAW(("dma", "v1"), v_dma, 1),
                       QK("k", 2, 0), QK("k", 2, 1), QK("k", 2, 2),
                       QK("q", 2, 0), QK("q", 2, 1)])
        pieces.extend(VC(1, ms) for ms in range(KS))
        pieces.extend([DMAW(("dma", "k3"), qk_dma, 3, "k"),
                       DMAW(("dma", "q3"), qk_dma, 3, "q"),
                       QK("k", 3, 0), QK("k", 3, 1), QK("k", 3, 2),
                       QK("q", 3, 0), QK("q", 3, 1),
                       DMAW(("dma", "k4"), qk_dma, 4, "k"),
                       DMAW(("dma", "q4"), qk_dma, 4, "q"),
                       QK("k", 4, 0), QK("k", 4, 1), QK("k", 4, 2),
                       QK("q", 4, 0), QK("q", 4, 1),
                       DMAW(("dma", "wo"),
                            lambda: nc.sync.dma_start(out=wo_s[:],
                                                      in_=wo_d[:]))])
        pieces.extend([QK("q", m, 2) for m in range(2, MD)])

        # unit sequence: chunk-0/1 units interleaved, ordered so units on
        # already-projected kT planes run first -- ACT exp saturates early
        # while the remaining k-plane/v projections drain behind it.
        seq = [(1, 0), (0, 1), (1, 1), (0, 2), (1, 2)]
        for m in range(1, MD - 1):
            seq += [(2 * m, 0), (2 * m, 1), (2 * m + 1, 0), (2 * m + 1, 1)]
        seq += [(8, 0), (9, 0), (8, 1), (9, 1)]
        seq += [(h, 2) for h in range(2, HPG)]
        budgets = {0: 1700, 1: 1800, 2: 2000}
        lean = [True] * 8

        for h, c in seq:
            b = budgets[c]
            if lean:
                b = 1000
                lean.pop()
            popped = unit(h, c, budget=b)
            if popped == (HPG - 1, 0):
                for qt in range(0, 4):
                    pieces.extend(OP(qt, ni) for ni in range(3))
            elif popped == (HPG - 1, 1):
                for qt in range(4, 8):
                    pieces.extend(OP(qt, ni) for ni in range(3))

        flush()                 # pv(7,2)
        for ni in range(3):     # reserved: rides out exp(8,2)/(9,2) latency
            oproj_chain(6, ni)
        flush()                 # pv(8,2)
        for ni in range(3):
            oproj_chain(7, ni)
        # last flush, fully pipelined per qtile: pv(qt) -> recip/norm (DVE)
        # -> PE transpose of qt-1 between pv pieces -> ACT copy + fp8 cast,
        # so the final o-proj's first dependencies land while pv still runs.
        lh, lc, lex, lcq = pending.popleft()
        lpc = pv_psum()
        for qt_i in range(4):
            pv_piece(lh, lc, lex, qt_i, lpc)
            zq = zpool.tile([128, 1, 1], dt.float32, name=f"lzq{qt_i}",
                            tag="zr")
            nc.vector.reciprocal(zq[:], lpc[:, qt_i:qt_i + 1, DH:DH + 1])
            nc.vector.tensor_tensor(
                lcq[:, qt_i, lh * DH:(lh + 1) * DH],
                lpc[:, qt_i, 0:DH],
                zq[:, 0].to_broadcast([128, DH]), op=ALU.mult)
            if qt_i >= 1:
                pst = psc.tile([128, MD, 128], dt.float16,
                               name=f"lpst{qt_i - 1}", tag="psc")
                for m in range(MD):
                    nc.tensor.transpose(
                        pst[:, m, :],
                        lcq[0:128, qt_i - 1, m * 128:(m + 1) * 128],
                        ident[:])
                qt = 8 + qt_i - 1
                ct = new_ct(qt)
                nc.scalar.copy(ct[:], pst[:])
                ct_cast(qt, fast=True)
        pst = psc.tile([128, MD, 128], dt.float16, name="lpst3", tag="psc")
        for m in range(MD):
            nc.tensor.transpose(
                pst[:, m, :], lcq[0:128, 3, m * 128:(m + 1) * 128], ident[:])
        ct = new_ct(11)
        nc.scalar.copy(ct[:], pst[:])
        ct_cast(11, fast=True)
        pop_budget(10**9)       # anything left
        for ni in range(3):
            for qt in range(8, 12):
                oproj_chain(qt, ni, act_copy=(qt == 11 and ni == 2),
                            dma_act=(qt % 2 == 1 and ni < 2))

    nc.compile()
    return nc


def _get_nc():
    if "nc" not in _CACHE:
        _CACHE["nc"] = build()
    return _CACHE["nc"]


def _split8(a):
    hi = a.astype(E4)
    lo = (a - hi.astype(np.float32)).astype(E4)
    return hi, lo


def _prep_in_maps(x, Wq, bq, Wk, Wv, Wo):
    in_maps = []
    for c in range(N_CORES):
        b, g = divmod(c, G)
        gs = slice(g * DG, (g + 1) * DG)
        # x planes: [kk, part, S] -> [part, KP, 2, S]
        xT = np.ascontiguousarray(x[b].T).astype(np.float32)
        xp = xT.reshape(KP, 2, 128, S).transpose(2, 0, 1, 3)
        xpad = np.zeros((128, KP, 2, SP_), dtype=np.float32)
        xpad[:, :, :, :S] = xp
        x8, dx = _split8(xpad)

        def wqk_prep(W):
            w = (WSCALE * W[gs, :].T).astype(np.float32)      # [D, DG]
            w = w.reshape(KD, 128, MD, 128)                   # [kk, part, m, col]
            w8, wd = _split8(w)
            # -> [MD, part, which, KP, 2, 128]
            def lay(a):
                return a.reshape(KP, 2, 128, MD, 128).transpose(3, 2, 0, 1, 4)
            return np.ascontiguousarray(
                np.stack([lay(w8), lay(wd)], axis=2))

        def wv_prep(W):
            w = (WSCALE * W[gs, :].T).astype(np.float32)
            w = w.reshape(KD, 128, G, 320)                    # [kk, part, n, col]
            w8, wd = _split8(w)
            def lay(a):
                return a.reshape(KP, 2, 128, G, 320).transpose(3, 2, 0, 1, 4)
            return np.ascontiguousarray(np.stack([lay(w8), lay(wd)], axis=2))

        # o-proj: [plane, part, col] blocks
        wo_ = (WSCALE * Wo[:, gs].T).astype(np.float32).reshape(MD, 128, D)
        wo8, wod = _split8(wo_)
        blocks = [
            np.stack([wo8[0], wo8[1]], axis=1),
            np.stack([wo8[2], wo8[3]], axis=1),
            np.stack([wod[0], wod[1]], axis=1),
            np.stack([wod[2], wod[3]], axis=1),
            np.stack([wo8[4], wo8[4]], axis=1),
            np.stack([wod[4], wod[4]], axis=1),
        ]
        wo_t = np.ascontiguousarray(
            np.stack(blocks, axis=0).transpose(1, 0, 2, 3))   # [128,6,2,D]

        in_maps.append({
            "x8": np.ascontiguousarray(x8),
            "dx": np.ascontiguousarray(dx),
            "wq": wqk_prep(Wq),
            "wk": wqk_prep(Wk),
            "wv": wv_prep(Wv),
            "wo": wo_t,
            "bq": np.ascontiguousarray(
                (0.125 * bq[gs]).astype(np.float32).reshape(MD, 128).T),
        })
    return in_maps


def run(x, Wq, bq, Wk, Wv, bv, Wo, bo, trace=False, **trace_kw):
    x = np.asarray(x, dtype=np.float32)
    Wq = np.asarray(Wq, dtype=np.float32)
    bq = np.asarray(bq, dtype=np.float32)
    Wk = np.asarray(Wk, dtype=np.float32)
    Wv = np.asarray(Wv, dtype=np.float32)
    bv = np.asarray(bv, dtype=np.float32)
    Wo = np.asarray(Wo, dtype=np.float32)
    bo = np.asarray(bo, dtype=np.float32)

    nc = _get_nc()
    in_maps = _prep_in_maps(x, Wq, bq, Wk, Wv, Wo)
    res = None
    for attempt in range(3):
        try:
            res = run_bass_kernel_spmd(nc, in_maps, list(range(N_CORES)),
                                       trace=trace, **trace_kw)
            break
        except Exception:
            # Sporadic NRT_EXEC_UNIT_UNRECOVERABLE on first exec; devices
            # come back after ~75s. Reset the backend and retry.
            if attempt == 2:
                raise
            import time as _time
            import jax as _jax
            _time.sleep(80)
            try:
                _jax.clear_backends()
            except Exception:
                pass
    const = (bv @ Wo.T + bo).astype(np.float32)  # [D]
    out = np.empty((B, S, D), dtype=np.float32)
    for b in range(B):
        out[b] = res.results[2 * b]["out"] + res.results[2 * b + 1]["out"] + const
    return out, res


def kernel(**inputs):
    out, _ = run(**inputs)
    return out


# revision 28
# speedup vs baseline: 1.0644x; 1.0082x over previous
"""Trainium2 Bass kernel: Whisper-style self-attention (B=4, S=1500, D=1280, H=20).

Sharding: core c = 2*b + g handles batch b (of 4) and head-group g (of 2,
10 heads each).  Q/K/V projections column-sharded over the head group,
attention sharded by (batch, head), output projection row-sharded; the two
head-group partials of each batch are summed on the host (plus bias terms).

v3 dataflow: projections run as fp8e4 DoubleRow matmuls (0.5 cycles/row,
2 contraction planes per instruction) with 3-term error compensation:
x ~ x8+dx, W ~ W8+dW (all e4m3, W pre-scaled x32 so residuals clear the
subnormal floor), computing x8W8 + x8dW + dxW8 -- ~fp16 accuracy at 0.75x
the fp16 PE cost for q/k/v and 0.8x for the o-projection (ctx split into
c8+dc on the gpsimd engine after the f16 transpose).  Scale bookkeeping:
q drain mult 0.125/32 (+0.125bq), k/v drains mult 1/32, PV "ones" column
1/16 (so ctxq = 16*ctx, putting the fp8 ctx split in range), o-proj drain
mult 1/512.  Scores/softmax/PV stay fp16 (fp8 attention weights flush to
zero below e4m3's subnormal floor and crater accuracy).

Scheduling: units are woven as [score-pair, pv-piece, score-pair, ...] with
the pv of unit u-2 riding between unit u's score pairs, so the PE never
waits on ACT exp draining the 2-buf score PSUM.  Projection / O-proj work
sits in a prerequisite-keyed FIFO of ~1-2us pieces; each unit first drains
the pieces its scores/pv depend on, then pops a tunable extra budget.
"""
import sys
sys.path.insert(0, "/opt/trn_rl_repo")

from collections import deque
from contextlib import ExitStack
import numpy as np
import ml_dtypes

import concourse.bass as bass
import concourse.tile as tile
from concourse import bacc, mybir
from concourse.bass_utils import run_bass_kernel_spmd

dt = mybir.dt
AF = mybir.ActivationFunctionType
ALU = mybir.AluOpType
PM = mybir.MatmulPerfMode
E4 = ml_dtypes.float8_e4m3

N_CORES = 8
B, S, D = 4, 1500, 1280
H, DH = 20, 64
G = 2
DG = D // G            # 640
HPG = H // G           # 10
KD = D // 128          # 10 contraction planes for D
KP = KD // 2           # 5 DoubleRow plane-pairs
MD = DG // 128         # 5 dh-planes per group
CW = (512, 512, 476)   # q/proj chunk widths
CO = (0, 512, 1024)
NS = 3
KS = (S + 127) // 128  # 12 k-tiles (11*128 + 92)
SP_ = 12 * 128         # 1536: padded S for ctxT columns
ON = (512, 512, 256)   # o-proj n chunks
WSCALE = 32.0          # fp8 weight pre-scale (subnormal headroom)
CSCALE = 16.0          # ctx pre-scale for fp8 split

_CACHE = {}


def _sk(i):
    return min(128, S - i * 128)


def build():
    nc = bacc.Bacc("TRN2", target_bir_lowering=False, debug=False,
                   num_devices=N_CORES)
    x8_d = nc.dram_tensor("x8", [128, KP, 2, SP_], dt.float8e4,
                          kind="ExternalInput").ap()
    dx_d = nc.dram_tensor("dx", [128, KP, 2, SP_], dt.float8e4,
                          kind="ExternalInput").ap()
    # [m, part, which(W8/dW), pair, parity, col]
    wq_d = nc.dram_tensor("wq", [MD, 128, 2, KP, 2, 128], dt.float8e4,
                          kind="ExternalInput").ap()
    wk_d = nc.dram_tensor("wk", [MD, 128, 2, KP, 2, 128], dt.float8e4,
                          kind="ExternalInput").ap()
    wv_d = nc.dram_tensor("wv", [G, 128, 2, KP, 2, 320], dt.float8e4,
                          kind="ExternalInput").ap()
    # o-proj rhs blocks: A=(W8p0,W8p1) B=(W8p2,W8p3) C=(dWp0,dWp1)
    # D=(dWp2,dWp3) E=(W8p4,W8p4) F=(dWp4,dWp4)
    wo_d = nc.dram_tensor("wo", [128, 6, 2, D], dt.float8e4,
                          kind="ExternalInput").ap()
    bq_d = nc.dram_tensor("bq", [128, MD], dt.float32, kind="ExternalInput").ap()
    out_d = nc.dram_tensor("out", [S, D], dt.float32, kind="ExternalOutput").ap()

    with tile.TileContext(nc) as tc, ExitStack() as octx:
        persist = octx.enter_context(tc.tile_pool(name="persist", bufs=1))
        wqkp = octx.enter_context(tc.tile_pool(name="wqk", bufs=6))
        wvp = octx.enter_context(tc.tile_pool(name="wv", bufs=2))
        epool = octx.enter_context(tc.tile_pool(name="expT", bufs=4))
        cqpool = octx.enter_context(tc.tile_pool(name="ctxq", bufs=3))
        zpool = octx.enter_context(tc.tile_pool(name="z", bufs=3))
        opool = octx.enter_context(tc.tile_pool(name="ob", bufs=3))
        ctpool = octx.enter_context(tc.tile_pool(name="ctxT", bufs=4))
        ps1 = octx.enter_context(tc.tile_pool(name="ps1", bufs=2, space="PSUM"))
        ps2 = octx.enter_context(tc.tile_pool(name="ps2", bufs=2, space="PSUM"))
        psc = octx.enter_context(tc.tile_pool(name="psc", bufs=2, space="PSUM"))

        def pv_psum():
            return psc.tile([128, 4, DH + 1], dt.float32,
                            name="pvps", tag="psc")

        x8_s = persist.tile([128, KP, 2, SP_], dt.float8e4, tag="x8")
        dx_s = persist.tile([128, KP, 2, SP_], dt.float8e4, tag="dx")
        qT = persist.tile([128, MD, S], dt.float16, tag="qT")
        kT = persist.tile([128, MD, S], dt.float16, tag="kT")
        v = persist.tile([128, KS, HPG, DH + 1], dt.float16, tag="v")
        cT2 = persist.tile([128, MD, 2, SP_], dt.float8e4, tag="cT2")
        wo_s = persist.tile([128, 6, 2, D], dt.float8e4, tag="wo")
        bq_s = persist.tile([128, MD], dt.float32, tag="bq")

        wts = {}

        def qk_dma(m, which):
            w_d = wq_d if which == "q" else wk_d
            wt = wqkp.tile([128, 2, KP, 2, 128], dt.float8e4,
                           name=f"w{which}{m}", tag="wqk")
            nc.sync.dma_start(out=wt[:], in_=w_d[m])
            wts[which, m] = wt

        def v_dma(n):
            wt = wvp.tile([128, 2, KP, 2, 320], dt.float8e4, name=f"wv{n}",
                          tag="wv")
            nc.sync.dma_start(out=wt[:], in_=wv_d[n])
            wts["v", n] = wt

        # startup DMA order is tuned so the PE preamble below never waits.
        qk_dma(0, "k")
        nc.sync.dma_start(out=x8_s[:, :, :, 0:128], in_=x8_d[:, :, :, 0:128])
        nc.sync.dma_start(out=dx_s[:, :, :, 0:128], in_=dx_d[:, :, :, 0:128])
        qk_dma(0, "q")
        nc.sync.dma_start(out=x8_s[:, :, :, 0:512], in_=x8_d[:, :, :, 0:512])
        nc.sync.dma_start(out=dx_s[:, :, :, 0:512], in_=dx_d[:, :, :, 0:512])
        nc.sync.dma_start(out=bq_s[:], in_=bq_d[:])
        v_dma(0)
        qk_dma(1, "k")
        qk_dma(1, "q")
        for co in range(512, S, 512):
            cw = min(512, S - co)
            nc.sync.dma_start(out=x8_s[:, :, :, co:co + cw],
                              in_=x8_d[:, :, :, co:co + cw])
            nc.sync.dma_start(out=dx_s[:, :, :, co:co + cw],
                              in_=dx_d[:, :, :, co:co + cw])

        from concourse.masks import make_identity
        ident = persist.tile([128, 128], dt.float16, tag="ident")
        make_identity(nc, ident[:])
        ones1 = persist.tile([128, 1], dt.float16, tag="ones1")
        nc.vector.memset(ones1[:], 1.0 / CSCALE)
        nc.vector.tensor_copy(v[:, :, :, DH:DH + 1],
                              ones1[:].to_broadcast([128, KS, HPG, 1]))

        # DoubleRow 3-term order: (x-src, w-sel) per plane-pair
        TERMS = ((0, 0), (1, 0), (0, 1))   # x8*W8, dx*W8, x8*dW

        def qk_chain(which, m, n, co=None, cw=None):
            """One projection chain: qT/kT plane m, column chunk n. ~1.6us."""
            wt = wts[which, m]
            dst = qT if which == "q" else kT
            if co is None:
                cw, co = CW[n], CO[n]
            ps = ps1.tile([128, 1, 512], dt.float32, tag="ps1")
            n_inst = len(TERMS) * KP
            i = 0
            for xsel, wsel in TERMS:
                xsrc = x8_s if xsel == 0 else dx_s
                for pp in range(KP):
                    nc.tensor.matmul(
                        ps[:, 0, 0:cw],
                        lhsT=wt[:, wsel, pp],
                        rhs=xsrc[:, pp, :, co:co + cw],
                        perf_mode=PM.DoubleRow,
                        start=(i == 0), stop=(i == n_inst - 1))
                    i += 1
            osl = dst[:, m, co:co + cw]
            if which == "q":
                nc.vector.tensor_scalar(
                    osl, ps[:, 0, 0:cw], 0.125 / WSCALE, bq_s[:, m:m + 1],
                    op0=ALU.mult, op1=ALU.add)
            else:
                nc.vector.tensor_scalar_mul(osl, ps[:, 0, 0:cw], 1.0 / WSCALE)

        def v_chain(n, ms):
            """v columns for heads 5n..5n+4, s-tile ms. ~1.0us."""
            wt = wts["v", n]
            sp = _sk(ms)
            ps = ps1.tile([128, 1, 512], dt.float32, tag="ps1")
            n_inst = len(TERMS) * KP
            i = 0
            for xsel, wsel in TERMS:
                xsrc = x8_s if xsel == 0 else dx_s
                for pp in range(KP):
                    nc.tensor.matmul(
                        ps[0:sp, 0, 0:320],
                        lhsT=xsrc[:, pp, :, ms * 128:ms * 128 + sp],
                        rhs=wt[:, wsel, pp],
                        perf_mode=PM.DoubleRow,
                        start=(i == 0), stop=(i == n_inst - 1))
                    i += 1
            nc.vector.tensor_scalar_mul(
                v[0:sp, ms, n * 5:(n + 1) * 5, 0:DH],
                ps[0:sp, 0, 0:320].rearrange("p (h e) -> p h e", h=5),
                1.0 / WSCALE)

        def ct_cast(qt, fast=False):
            """ctxT f16 [128, MD, 128] for q-tile qt -> cT2 fp8 (c8, dc).
            The two passes split across engines so consecutive q-tiles
            pipeline instead of serializing on gpsimd."""
            ct = ct_by_qt.pop(qt)
            qo = qt * 128
            ceng, seng = (nc.vector, nc.gpsimd) if fast else (nc.gpsimd,
                                                              nc.vector)
            ceng.tensor_copy(cT2[:, :, 0, qo:qo + 128], ct[:])
            seng.tensor_tensor(
                cT2[:, :, 1, qo:qo + 128], ct[:],
                cT2[:, :, 0, qo:qo + 128], op=ALU.subtract)

        def oproj_chain(qt, ni, act_copy=False, dma_act=False):
            """One o-proj chain: q-tile qt, n-chunk ni (8 DoubleRow insts).
            act_copy routes the PSUM drain to the ACT engine (idle at the
            kernel tail) so it can't delay DVE's critical recip/norm."""
            sp = _sk(qt)
            mw = 128 if sp == 128 else 96
            nw = ON[ni]
            noff = CO[ni]
            qo = qt * 128
            ps = ps1.tile([128, 1, 512], dt.float32, tag="ps1")
            insts = [
                (cT2[:, 0:2, 0, qo:qo + mw], 0),
                (cT2[:, 2:4, 0, qo:qo + mw], 1),
                (cT2[:, 0:2, 1, qo:qo + mw], 0),
                (cT2[:, 2:4, 1, qo:qo + mw], 1),
                (cT2[:, 0:2, 0, qo:qo + mw], 2),
                (cT2[:, 2:4, 0, qo:qo + mw], 3),
                (cT2[:, 4, 0:2, qo:qo + mw], 4),
                (cT2[:, 4, 0:2, qo:qo + mw], 5),
            ]
            for i, (lh, wb) in enumerate(insts):
                nc.tensor.matmul(
                    ps[0:mw, 0, 0:nw],
                    lhsT=lh,
                    rhs=wo_s[:, wb, :, noff:noff + nw],
                    perf_mode=PM.DoubleRow,
                    start=(i == 0), stop=(i == len(insts) - 1))
            ob = opool.tile([128, 512], dt.float32, tag="ob")
            if act_copy:
                nc.scalar.mul(ob[0:sp, 0:nw], ps[0:sp, 0, 0:nw],
                              1.0 / (WSCALE * CSCALE))
            else:
                nc.vector.tensor_scalar_mul(ob[0:sp, 0:nw], ps[0:sp, 0, 0:nw],
                                            1.0 / (WSCALE * CSCALE))
            deng = nc.scalar if dma_act else nc.sync
            deng.dma_start(
                out=out_d[qt * 128:qt * 128 + sp, noff:noff + nw],
                in_=ob[0:sp, 0:nw])

        # ---- prerequisite-keyed filler piece queue --------------------
        pieces = deque()        # (key, fn, cost_ns)
        emitted = set()
        emitted.update([("k", 0, 0), ("k", 0, 1), ("k", 0, 2), ("q", 0, 0)])

        def pop_one():
            key, fn, cost = pieces.popleft()
            fn()
            emitted.add(key)
            return cost

        def drain_until(keys):
            need = [k for k in keys if k not in emitted]
            for k in need:
                while k not in emitted:
                    assert pieces, f"piece schedule missing prerequisite {k}"
                    pop_one()

        def pop_budget(budget):
            while budget > 0 and pieces:
                budget -= pop_one()

        def QK(which, m, n):
            return ((which, m, n), lambda: qk_chain(which, m, n), 1650)

        def VC(n, ms):
            return (("v", n, ms), lambda: v_chain(n, ms), 1050)

        def DMAW(key, fn, *a):
            return (key, lambda: fn(*a), 50)

        def OP(qt, ni):
            return (("op", qt, ni), lambda: oproj_chain(qt, ni), 900)

        def CC(qt):
            return (("cc", qt), lambda: ct_cast(qt), 100)

        # ---- attention unit, woven ------------------------------------
        pending = deque()       # (h, c, ex, ctxq_tile) awaiting pv
        ctxq_by_c = {}
        ct_by_qt = {}

        def pv_piece(ph, pc_, pex, qt_i, pc_t):
            cw = CW[pc_]
            qco = qt_i * 128
            qw = min(128, cw - qco)
            for kk in range(KS):
                sp = _sk(kk)
                nc.tensor.matmul(
                    pc_t[0:qw, qt_i, :],
                    lhsT=pex[0:sp, kk, qco:qco + qw],
                    rhs=v[0:sp, kk, ph, :],
                    start=(kk == 0), stop=(kk == KS - 1))

        def new_ct(qt):
            ct = ctpool.tile([128, MD, 128], dt.float16, name=f"ct{qt}",
                             tag="ctxT")
            ct_by_qt[qt] = ct
            return ct

        def pv_finish(ph, pc_, pcq, pc_t):
            last = ph == HPG - 1
            if last and pc_ == 2:
                # end-game: per-qtile recip+norm on DVE, then PE transposes
                # (fp16 PSUM) + ACT copies into per-qtile ctxT tiles.
                for qt_i in range(4):
                    zq = zpool.tile([128, 1, 1], dt.float32, name=f"zq{qt_i}",
                                    tag="zr")
                    nc.vector.reciprocal(zq[:], pc_t[:, qt_i:qt_i + 1,
                                                     DH:DH + 1])
                    nc.vector.tensor_tensor(
                        pcq[:, qt_i, ph * DH:(ph + 1) * DH],
                        pc_t[:, qt_i, 0:DH],
                        zq[:, 0].to_broadcast([128, DH]), op=ALU.mult)
                psts = []
                for qt_i in range(4):
                    pst = psc.tile([128, MD, 128], dt.float16,
                                   name=f"pst{qt_i}", tag="psc")
                    for m in range(MD):
                        nc.tensor.transpose(
                            pst[:, m, :],
                            pcq[0:128, qt_i, m * 128:(m + 1) * 128],
                            ident[:])
                    psts.append(pst)
                for qt_i in range(4):
                    qt = 4 * pc_ + qt_i
                    ct = new_ct(qt)
                    nc.scalar.copy(ct[:], psts[qt_i][:])
                    ct_cast(qt, fast=True)
                return
            zr = zpool.tile([128, 4, 1], dt.float32, tag="zr")
            nc.vector.reciprocal(zr[:], pc_t[:, :, DH:DH + 1])
            nc.vector.tensor_tensor(
                pcq[:, :, ph * DH:(ph + 1) * DH], pc_t[:, :, 0:DH],
                zr[:].to_broadcast([128, 4, DH]), op=ALU.mult)
            if last:
                for qt_i in range(4):
                    qt = 4 * pc_ + qt_i
                    pp = 128 if _sk(qt) == 128 else 96
                    ct = new_ct(qt)
                    nc.sync.dma_start_transpose(
                        out=ct[:, :, 0:pp],
                        in_=pcq[0:pp, qt_i, :])
                    pieces.appendleft(CC(qt))

        def get_ctxq(c):
            if c not in ctxq_by_c:
                ctxq_by_c[c] = cqpool.tile([128, 4, DG], dt.float16,
                                           name=f"ctxq{c}", tag="ctxq")
            return ctxq_by_c[c]

        def scores_pair(h, c, kk2, ex):
            base = 64 * (h % 2)
            td = h // 2
            cw, co = CW[c], CO[c]
            ps = ps2.tile([128, 2, 512], dt.float32, tag="ps2")
            for j in range(2):
                kk = kk2 + j
                sp = _sk(kk)
                nc.tensor.matmul(
                    ps[0:sp, j, 0:cw],
                    lhsT=kT[base:base + 64, td, kk * 128:kk * 128 + sp],
                    rhs=qT[base:base + 64, td, co:co + cw],
                    start=True, stop=True)
            nc.scalar.activation(ex[:, kk2:kk2 + 2, 0:cw], ps[:, :, 0:cw],
                                 AF.Exp)

        slot_no = [0]

        def unit(h, c, budget=2200):
            get_ctxq(c)
            m = h // 2
            prereq = [("k", m, 0), ("k", m, 1), ("k", m, 2), ("q", m, c)]
            lag = 3 if slot_no[0] < 5 else 2
            slot_no[0] += 1
            do_pv = len(pending) >= lag
            if do_pv:
                ph = pending[0][0]
                prereq += [("v", ph // 5, ms) for ms in range(KS)]
            drain_until(prereq)
            popped = None
            if do_pv:
                ph, pc_, pex, pcq = pending.popleft()
                popped = (ph, pc_)
                pc_t = pv_psum()
            ex = epool.tile([128, KS, 512], dt.float16, tag="expT")
            for kk2 in range(0, KS, 2):
                scores_pair(h, c, kk2, ex)
                if do_pv and kk2 >= 4:      # weave pv qtiles btwn pairs 3..6
                    pv_piece(ph, pc_, pex, kk2 // 2 - 2, pc_t)
            if do_pv:
                pv_finish(ph, pc_, pcq, pc_t)
            pending.append((h, c, ex, ctxq_by_c[c]))
            pop_budget(budget)
            return popped

        def flush():
            ph, pc_, pex, pcq = pending.popleft()
            drain_until([("v", ph // 5, ms) for ms in range(KS)])
            pc_t = pv_psum()
            for qt_i in range(4):
                if pieces:
                    pop_one()
                pv_piece(ph, pc_, pex, qt_i, pc_t)
            pv_finish(ph, pc_, pcq, pc_t)

        # ---- preamble: unit(0,0)'s score pairs hand-woven between the
        # projection chains they depend on, so ACT exp starts early.
        qk_chain("k", 0, 0, co=0, cw=128)
        qk_chain("q", 0, 0, co=0, cw=128)
        qk_chain("k", 0, 0, co=128, cw=384)
        qk_chain("q", 0, 0, co=128, cw=384)
        ex0 = epool.tile([128, KS, 512], dt.float16, tag="expT")
        scores_pair(0, 0, 0, ex0)
        scores_pair(0, 0, 2, ex0)
        v_chain(0, 0)
        v_chain(0, 1)
        qk_chain("k", 1, 0)
        qk_chain("q", 1, 0)
        qk_chain("k", 0, 1)
        scores_pair(0, 0, 4, ex0)
        scores_pair(0, 0, 6, ex0)
        v_chain(0, 2)
        v_chain(0, 3)
        qk_chain("k", 1, 1)
        qk_chain("k", 0, 2)
        scores_pair(0, 0, 8, ex0)
        scores_pair(0, 0, 10, ex0)
        pending.append((0, 0, ex0, get_ctxq(0)))
        emitted.update([("k", 1, 0), ("k", 1, 1), ("q", 1, 0)] +
                       [("v", 0, i) for i in range(4)])

        # piece FIFO in first-use order (see unit sequence below).
        pieces.extend([QK("q", 0, 1), QK("q", 1, 1), QK("k", 1, 2),
                       QK("q", 0, 2), QK("q", 1, 2)])
        pieces.extend(VC(0, ms) for ms in range(4, KS))
        pieces.extend([DMAW(("dma", "k2"), qk_dma, 2, "k"),
                       DMAW(("dma", "q2"), qk_dma, 2, "q"),
                       DMAW(("dma", "v1"), v_dma, 1),
                       QK("k", 2, 0), QK("k", 2, 1), QK("k", 2, 2),
                       QK("q", 2, 0), QK("q", 2, 1)])
        pieces.extend(VC(1, ms) for ms in range(KS))
        pieces.extend([DMAW(("dma", "k3"), qk_dma, 3, "k"),
                       DMAW(("dma", "q3"), qk_dma, 3, "q"),
                       QK("k", 3, 0), QK("k", 3, 1), QK("k", 3, 2),
                       QK("q", 3, 0), QK("q", 3, 1),
                       DMAW(("dma", "k4"), qk_dma, 4, "k"),
                       DMAW(("dma", "q4"), qk_dma, 4, "q"),
                       QK("k", 4, 0), QK("k", 4, 1), QK("k", 4, 2),
                       QK("q", 4, 0), QK("q", 4, 1),
                       DMAW(("dma", "wo"),
                            lambda: nc.sync.dma_start(out=wo_s[:],
                                                      in_=wo_d[:]))])
        pieces.extend([QK("q", m, 2) for m in range(2, MD)])

        # unit sequence: chunk-0/1 units interleaved, ordered so units on
        # already-projected kT planes run first -- ACT exp saturates early
        # while the remaining k-plane/v projections drain behind it.
        seq = [(1, 0), (0, 1), (1, 1), (0, 2), (1, 2)]
        for m in range(1, MD - 1):
            seq += [(2 * m, 0), (2 * m, 1), (2 * m + 1, 0), (2 * m + 1, 1)]
        seq += [(8, 0), (9, 0), (8, 1), (9, 1)]
        seq += [(h, 2) for h in range(2, HPG)]
        budgets = {0: 1700, 1: 1800, 2: 2000}
        lean = [True] * 8

        for h, c in seq:
            b = budgets[c]
            if lean:
                b = 1000
                lean.pop()
            popped = unit(h, c, budget=b)
            if popped == (HPG - 1, 0):
                for qt in range(0, 4):
                    pieces.extend(OP(qt, ni) for ni in range(3))
            elif popped == (HPG - 1, 1):
                for qt in range(4, 6):   # qt 6/7 are emitted inline at the
                    pieces.extend(OP(qt, ni) for ni in range(3))  # tail

        flush()                 # pv(7,2)
        for ni in range(3):     # reserved: rides out exp(8,2)/(9,2) latency
            oproj_chain(6, ni)
        flush()                 # pv(8,2)
        for ni in range(3):
            oproj_chain(7, ni)
        # last flush, fully pipelined per qtile: pv(qt) -> recip/norm (DVE)
        # -> PE transpose of qt-1 between pv pieces -> ACT copy + fp8 cast,
        # so the final o-proj's first dependencies land while pv still runs.
        lh, lc, lex, lcq = pending.popleft()
        lpc = pv_psum()
        for qt_i in range(4):
            pv_piece(lh, lc, lex, qt_i, lpc)
            zq = zpool.tile([128, 1, 1], dt.float32, name=f"lzq{qt_i}",
                            tag="zr")
            nc.vector.reciprocal(zq[:], lpc[:, qt_i:qt_i + 1, DH:DH + 1])
            nc.vector.tensor_tensor(
                lcq[:, qt_i, lh * DH:(lh + 1) * DH],
                lpc[:, qt_i, 0:DH],
                zq[:, 0].to_broadcast([128, DH]), op=ALU.mult)
            if qt_i >= 1:
                pst = psc.tile([128, MD, 128], dt.float16,
                               name=f"lpst{qt_i - 1}", tag="psc")
                for m in range(MD):
                    nc.tensor.transpose(
                        pst[:, m, :],
                        lcq[0:128, qt_i - 1, m * 128:(m + 1) * 128],
                        ident[:])
                qt = 8 + qt_i - 1
                ct = new_ct(qt)
                nc.scalar.copy(ct[:], pst[:])
                ct_cast(qt, fast=True)
                oproj_chain(qt, 0, act_copy=True)
        pst = psc.tile([128, MD, 128], dt.float16, name="lpst3", tag="psc")
        for m in range(MD):
            nc.tensor.transpose(
                pst[:, m, :], lcq[0:128, 3, m * 128:(m + 1) * 128], ident[:])
        ct = new_ct(11)
        nc.scalar.copy(ct[:], pst[:])
        ct_cast(11, fast=True)
        pop_budget(10**9)       # anything left
        for ni in (1, 2):
            for qt in range(8, 12):
                oproj_chain(qt, ni, act_copy=True,
                            dma_act=(qt % 2 == 1 and ni < 2))
        oproj_chain(11, 0, act_copy=True)

    nc.compile()
    return nc


def _get_nc():
    if "nc" not in _CACHE:
        _CACHE["nc"] = build()
    return _CACHE["nc"]


def _split8(a):
    hi = a.astype(E4)
    lo = (a - hi.astype(np.float32)).astype(E4)
    return hi, lo


def _prep_in_maps(x, Wq, bq, Wk, Wv, Wo):
    in_maps = []
    for c in range(N_CORES):
        b, g = divmod(c, G)
        gs = slice(g * DG, (g + 1) * DG)
        # x planes: [kk, part, S] -> [part, KP, 2, S]
        xT = np.ascontiguousarray(x[b].T).astype(np.float32)
        xp = xT.reshape(KP, 2, 128, S).transpose(2, 0, 1, 3)
        xpad = np.zeros((128, KP, 2, SP_), dtype=np.float32)
        xpad[:, :, :, :S] = xp
        x8, dx = _split8(xpad)

        def wqk_prep(W):
            w = (WSCALE * W[gs, :].T).astype(np.float32)      # [D, DG]
            w = w.reshape(KD, 128, MD, 128)                   # [kk, part, m, col]
            w8, wd = _split8(w)
            # -> [MD, part, which, KP, 2, 128]
            def lay(a):
                return a.reshape(KP, 2, 128, MD, 128).transpose(3, 2, 0, 1, 4)
            return np.ascontiguousarray(
                np.stack([lay(w8), lay(wd)], axis=2))

        def wv_prep(W):
            w = (WSCALE * W[gs, :].T).astype(np.float32)
            w = w.reshape(KD, 128, G, 320)                    # [kk, part, n, col]
            w8, wd = _split8(w)
            def lay(a):
                return a.reshape(KP, 2, 128, G, 320).transpose(3, 2, 0, 1, 4)
            return np.ascontiguousarray(np.stack([lay(w8), lay(wd)], axis=2))

        # o-proj: [plane, part, col] blocks
        wo_ = (WSCALE * Wo[:, gs].T).astype(np.float32).reshape(MD, 128, D)
        wo8, wod = _split8(wo_)
        blocks = [
            np.stack([wo8[0], wo8[1]], axis=1),
            np.stack([wo8[2], wo8[3]], axis=1),
            np.stack([wod[0], wod[1]], axis=1),
            np.stack([wod[2], wod[3]], axis=1),
            np.stack([wo8[4], wo8[4]], axis=1),
            np.stack([wod[4], wod[4]], axis=1),
        ]
        wo_t = np.ascontiguousarray(
            np.stack(blocks, axis=0).transpose(1, 0, 2, 3))   # [128,6,2,D]

        in_maps.append({
            "x8": np.ascontiguousarray(x8),
            "dx": np.ascontiguousarray(dx),
            "wq": wqk_prep(Wq),
            "wk": wqk_prep(Wk),
            "wv": wv_prep(Wv),
            "wo": wo_t,
            "bq": np.ascontiguousarray(
                (0.125 * bq[gs]).astype(np.float32).reshape(MD, 128).T),
        })
    return in_maps


def run(x, Wq, bq, Wk, Wv, bv, Wo, bo, trace=False, **trace_kw):
    x = np.asarray(x, dtype=np.float32)
    Wq = np.asarray(Wq, dtype=np.float32)
    bq = np.asarray(bq, dtype=np.float32)
    Wk = np.asarray(Wk, dtype=np.float32)
    Wv = np.asarray(Wv, dtype=np.float32)
    bv = np.asarray(bv, dtype=np.float32)
    Wo = np.asarray(Wo, dtype=np.float32)
    bo = np.asarray(bo, dtype=np.float32)

    nc = _get_nc()
    in_maps = _prep_in_maps(x, Wq, bq, Wk, Wv, Wo)
    res = None
    for attempt in range(3):
        try:
            res = run_bass_kernel_spmd(nc, in_maps, list(range(N_CORES)),
                                       trace=trace, **trace_kw)
            break
        except Exception:
            # Sporadic NRT_EXEC_UNIT_UNRECOVERABLE on first exec; devices
            # come back after ~75s. Reset the backend and retry.
            if attempt == 2:
                raise
            import time as _time
            import jax as _jax
            _time.sleep(80)
            try:
                _jax.clear_backends()
            except Exception:
                pass
    const = (bv @ Wo.T + bo).astype(np.float32)  # [D]
    out = np.empty((B, S, D), dtype=np.float32)
    for b in range(B):
        out[b] = res.results[2 * b]["out"] + res.results[2 * b + 1]["out"] + const
    return out, res


def kernel(**inputs):
    out, _ = run(**inputs)
    return out


# revision 29
# speedup vs baseline: 1.0773x; 1.0122x over previous
"""Trainium2 Bass kernel: Whisper-style self-attention (B=4, S=1500, D=1280, H=20).

Sharding: core c = 2*b + g handles batch b (of 4) and head-group g (of 2,
10 heads each).  Q/K/V projections column-sharded over the head group,
attention sharded by (batch, head), output projection row-sharded; the two
head-group partials of each batch are summed on the host (plus bias terms).

v3 dataflow: projections run as fp8e4 DoubleRow matmuls (0.5 cycles/row,
2 contraction planes per instruction) with 3-term error compensation:
x ~ x8+dx, W ~ W8+dW (all e4m3, W pre-scaled x32 so residuals clear the
subnormal floor), computing x8W8 + x8dW + dxW8 -- ~fp16 accuracy at 0.75x
the fp16 PE cost for q/k/v and 0.8x for the o-projection (ctx split into
c8+dc on the gpsimd engine after the f16 transpose).  Scale bookkeeping:
q drain mult 0.125/32 (+0.125bq), k/v drains mult 1/32, PV "ones" column
1/16 (so ctxq = 16*ctx, putting the fp8 ctx split in range), o-proj drain
mult 1/512.  Scores/softmax/PV stay fp16 (fp8 attention weights flush to
zero below e4m3's subnormal floor and crater accuracy).

Scheduling: units are woven as [score-pair, pv-piece, score-pair, ...] with
the pv of unit u-2 riding between unit u's score pairs, so the PE never
waits on ACT exp draining the 2-buf score PSUM.  Projection / O-proj work
sits in a prerequisite-keyed FIFO of ~1-2us pieces; each unit first drains
the pieces its scores/pv depend on, then pops a tunable extra budget.
"""
import sys
sys.path.insert(0, "/opt/trn_rl_repo")

from collections import deque
from contextlib import ExitStack
import numpy as np
import ml_dtypes

import concourse.bass as bass
import concourse.tile as tile
from concourse import bacc, mybir
from concourse.bass_utils import run_bass_kernel_spmd

dt = mybir.dt
AF = mybir.ActivationFunctionType
ALU = mybir.AluOpType
PM = mybir.MatmulPerfMode
E4 = ml_dtypes.float8_e4m3

N_CORES = 8
B, S, D = 4, 1500, 1280
H, DH = 20, 64
G = 2
DG = D // G            # 640
HPG = H // G           # 10
KD = D // 128          # 10 contraction planes for D
KP = KD // 2           # 5 DoubleRow plane-pairs
MD = DG // 128         # 5 dh-planes per group
CW = (512, 512, 476)   # q/proj chunk widths
CO = (0, 512, 1024)
NS = 3
KS = (S + 127) // 128  # 12 k-tiles (11*128 + 92)
SP_ = 12 * 128         # 1536: padded S for ctxT columns
ON = (512, 512, 256)   # o-proj n chunks
WSCALE = 32.0          # fp8 weight pre-scale (subnormal headroom)
CSCALE = 16.0          # ctx pre-scale for fp8 split

_CACHE = {}


def _sk(i):
    return min(128, S - i * 128)


def build():
    nc = bacc.Bacc("TRN2", target_bir_lowering=False, debug=False,
                   num_devices=N_CORES)
    x8_d = nc.dram_tensor("x8", [128, KP, 2, SP_], dt.float8e4,
                          kind="ExternalInput").ap()
    dx_d = nc.dram_tensor("dx", [128, KP, 2, SP_], dt.float8e4,
                          kind="ExternalInput").ap()
    # [m, part, which(W8/dW), pair, parity, col]
    wq_d = nc.dram_tensor("wq", [MD, 128, 2, KP, 2, 128], dt.float8e4,
                          kind="ExternalInput").ap()
    wk_d = nc.dram_tensor("wk", [MD, 128, 2, KP, 2, 128], dt.float8e4,
                          kind="ExternalInput").ap()
    wv_d = nc.dram_tensor("wv", [G, 128, 2, KP, 2, 320], dt.float8e4,
                          kind="ExternalInput").ap()
    # o-proj rhs blocks: A=(W8p0,W8p1) B=(W8p2,W8p3) C=(dWp0,dWp1)
    # D=(dWp2,dWp3) E=(W8p4,W8p4) F=(dWp4,dWp4)
    wo_d = nc.dram_tensor("wo", [128, 6, 2, D], dt.float8e4,
                          kind="ExternalInput").ap()
    bq_d = nc.dram_tensor("bq", [128, MD], dt.float32, kind="ExternalInput").ap()
    out_d = nc.dram_tensor("out", [S, D], dt.float32, kind="ExternalOutput").ap()

    with tile.TileContext(nc) as tc, ExitStack() as octx:
        persist = octx.enter_context(tc.tile_pool(name="persist", bufs=1))
        wqkp = octx.enter_context(tc.tile_pool(name="wqk", bufs=6))
        wvp = octx.enter_context(tc.tile_pool(name="wv", bufs=2))
        epool = octx.enter_context(tc.tile_pool(name="expT", bufs=4))
        cqpool = octx.enter_context(tc.tile_pool(name="ctxq", bufs=3))
        zpool = octx.enter_context(tc.tile_pool(name="z", bufs=3))
        opool = octx.enter_context(tc.tile_pool(name="ob", bufs=3))
        ctpool = octx.enter_context(tc.tile_pool(name="ctxT", bufs=4))
        ps1 = octx.enter_context(tc.tile_pool(name="ps1", bufs=2, space="PSUM"))
        ps2 = octx.enter_context(tc.tile_pool(name="ps2", bufs=2, space="PSUM"))
        psc = octx.enter_context(tc.tile_pool(name="psc", bufs=2, space="PSUM"))

        def pv_psum():
            return psc.tile([128, 4, DH + 1], dt.float32,
                            name="pvps", tag="psc")

        x8_s = persist.tile([128, KP, 2, SP_], dt.float8e4, tag="x8")
        dx_s = persist.tile([128, KP, 2, SP_], dt.float8e4, tag="dx")
        qT = persist.tile([128, MD, S], dt.float16, tag="qT")
        kT = persist.tile([128, MD, S], dt.float16, tag="kT")
        v = persist.tile([128, KS, HPG, DH + 1], dt.float16, tag="v")
        cT2 = persist.tile([128, MD, 2, SP_], dt.float8e4, tag="cT2")
        wo_s = persist.tile([128, 6, 2, D], dt.float8e4, tag="wo")
        bq_s = persist.tile([128, MD], dt.float32, tag="bq")

        wts = {}

        def qk_dma(m, which):
            w_d = wq_d if which == "q" else wk_d
            wt = wqkp.tile([128, 2, KP, 2, 128], dt.float8e4,
                           name=f"w{which}{m}", tag="wqk")
            nc.sync.dma_start(out=wt[:], in_=w_d[m])
            wts[which, m] = wt

        def v_dma(n):
            wt = wvp.tile([128, 2, KP, 2, 320], dt.float8e4, name=f"wv{n}",
                          tag="wv")
            nc.sync.dma_start(out=wt[:], in_=wv_d[n])
            wts["v", n] = wt

        # startup DMA order is tuned so the PE preamble below never waits.
        qk_dma(0, "k")
        nc.sync.dma_start(out=x8_s[:, :, :, 0:512], in_=x8_d[:, :, :, 0:512])
        nc.sync.dma_start(out=dx_s[:, :, :, 0:512], in_=dx_d[:, :, :, 0:512])
        qk_dma(0, "q")
        nc.sync.dma_start(out=bq_s[:], in_=bq_d[:])
        v_dma(0)
        qk_dma(1, "k")
        qk_dma(1, "q")
        for co in range(512, S, 512):
            cw = min(512, S - co)
            nc.sync.dma_start(out=x8_s[:, :, :, co:co + cw],
                              in_=x8_d[:, :, :, co:co + cw])
            nc.sync.dma_start(out=dx_s[:, :, :, co:co + cw],
                              in_=dx_d[:, :, :, co:co + cw])

        from concourse.masks import make_identity
        ident = persist.tile([128, 128], dt.float16, tag="ident")
        make_identity(nc, ident[:])
        ones1 = persist.tile([128, 1], dt.float16, tag="ones1")
        nc.vector.memset(ones1[:], 1.0 / CSCALE)
        nc.vector.tensor_copy(v[:, :, :, DH:DH + 1],
                              ones1[:].to_broadcast([128, KS, HPG, 1]))

        # DoubleRow 3-term order: (x-src, w-sel) per plane-pair
        TERMS = ((0, 0), (1, 0), (0, 1))   # x8*W8, dx*W8, x8*dW

        def qk_chain(which, m, n, co=None, cw=None):
            """One projection chain: qT/kT plane m, column chunk n. ~1.6us."""
            wt = wts[which, m]
            dst = qT if which == "q" else kT
            if co is None:
                cw, co = CW[n], CO[n]
            ps = ps1.tile([128, 1, 512], dt.float32, tag="ps1")
            n_inst = len(TERMS) * KP
            i = 0
            for xsel, wsel in TERMS:
                xsrc = x8_s if xsel == 0 else dx_s
                for pp in range(KP):
                    nc.tensor.matmul(
                        ps[:, 0, 0:cw],
                        lhsT=wt[:, wsel, pp],
                        rhs=xsrc[:, pp, :, co:co + cw],
                        perf_mode=PM.DoubleRow,
                        start=(i == 0), stop=(i == n_inst - 1))
                    i += 1
            osl = dst[:, m, co:co + cw]
            if which == "q":
                nc.vector.tensor_scalar(
                    osl, ps[:, 0, 0:cw], 0.125 / WSCALE, bq_s[:, m:m + 1],
                    op0=ALU.mult, op1=ALU.add)
            else:
                nc.vector.tensor_scalar_mul(osl, ps[:, 0, 0:cw], 1.0 / WSCALE)

        def v_chain(n, ms):
            """v columns for heads 5n..5n+4, s-tile ms. ~1.0us."""
            wt = wts["v", n]
            sp = _sk(ms)
            ps = ps1.tile([128, 1, 512], dt.float32, tag="ps1")
            n_inst = len(TERMS) * KP
            i = 0
            for xsel, wsel in TERMS:
                xsrc = x8_s if xsel == 0 else dx_s
                for pp in range(KP):
                    nc.tensor.matmul(
                        ps[0:sp, 0, 0:320],
                        lhsT=xsrc[:, pp, :, ms * 128:ms * 128 + sp],
                        rhs=wt[:, wsel, pp],
                        perf_mode=PM.DoubleRow,
                        start=(i == 0), stop=(i == n_inst - 1))
                    i += 1
            nc.vector.tensor_scalar_mul(
                v[0:sp, ms, n * 5:(n + 1) * 5, 0:DH],
                ps[0:sp, 0, 0:320].rearrange("p (h e) -> p h e", h=5),
                1.0 / WSCALE)

        def ct_cast(qt, fast=False):
            """ctxT f16 [128, MD, 128] for q-tile qt -> cT2 fp8 (c8, dc).
            The two passes split across engines so consecutive q-tiles
            pipeline instead of serializing on gpsimd."""
            ct = ct_by_qt.pop(qt)
            qo = qt * 128
            ceng, seng = (nc.vector, nc.gpsimd) if fast else (nc.gpsimd,
                                                              nc.vector)
            ceng.tensor_copy(cT2[:, :, 0, qo:qo + 128], ct[:])
            seng.tensor_tensor(
                cT2[:, :, 1, qo:qo + 128], ct[:],
                cT2[:, :, 0, qo:qo + 128], op=ALU.subtract)

        def oproj_chain(qt, ni, act_copy=False, dma_act=False):
            """One o-proj chain: q-tile qt, n-chunk ni (8 DoubleRow insts).
            act_copy routes the PSUM drain to the ACT engine (idle at the
            kernel tail) so it can't delay DVE's critical recip/norm."""
            sp = _sk(qt)
            mw = 128 if sp == 128 else 96
            nw = ON[ni]
            noff = CO[ni]
            qo = qt * 128
            ps = ps1.tile([128, 1, 512], dt.float32, tag="ps1")
            insts = [
                (cT2[:, 0:2, 0, qo:qo + mw], 0),
                (cT2[:, 2:4, 0, qo:qo + mw], 1),
                (cT2[:, 0:2, 1, qo:qo + mw], 0),
                (cT2[:, 2:4, 1, qo:qo + mw], 1),
                (cT2[:, 0:2, 0, qo:qo + mw], 2),
                (cT2[:, 2:4, 0, qo:qo + mw], 3),
                (cT2[:, 4, 0:2, qo:qo + mw], 4),
                (cT2[:, 4, 0:2, qo:qo + mw], 5),
            ]
            for i, (lh, wb) in enumerate(insts):
                nc.tensor.matmul(
                    ps[0:mw, 0, 0:nw],
                    lhsT=lh,
                    rhs=wo_s[:, wb, :, noff:noff + nw],
                    perf_mode=PM.DoubleRow,
                    start=(i == 0), stop=(i == len(insts) - 1))
            ob = opool.tile([128, 512], dt.float32, tag="ob")
            if act_copy:
                nc.scalar.mul(ob[0:sp, 0:nw], ps[0:sp, 0, 0:nw],
                              1.0 / (WSCALE * CSCALE))
            else:
                nc.vector.tensor_scalar_mul(ob[0:sp, 0:nw], ps[0:sp, 0, 0:nw],
                                            1.0 / (WSCALE * CSCALE))
            deng = nc.scalar if dma_act else nc.sync
            deng.dma_start(
                out=out_d[qt * 128:qt * 128 + sp, noff:noff + nw],
                in_=ob[0:sp, 0:nw])

        # ---- prerequisite-keyed filler piece queue --------------------
        pieces = deque()        # (key, fn, cost_ns)
        emitted = set()
        emitted.update([("k", 0, 0), ("k", 0, 1), ("k", 0, 2), ("q", 0, 0)])

        def pop_one():
            key, fn, cost = pieces.popleft()
            fn()
            emitted.add(key)
            return cost

        def drain_until(keys):
            need = [k for k in keys if k not in emitted]
            for k in need:
                while k not in emitted:
                    assert pieces, f"piece schedule missing prerequisite {k}"
                    pop_one()

        def pop_budget(budget):
            while budget > 0 and pieces:
                budget -= pop_one()

        def QK(which, m, n):
            return ((which, m, n), lambda: qk_chain(which, m, n), 1650)

        def VC(n, ms):
            return (("v", n, ms), lambda: v_chain(n, ms), 1050)

        def DMAW(key, fn, *a):
            return (key, lambda: fn(*a), 50)

        def OP(qt, ni):
            return (("op", qt, ni), lambda: oproj_chain(qt, ni), 900)

        def CC(qt):
            return (("cc", qt), lambda: ct_cast(qt), 100)

        # ---- attention unit, woven ------------------------------------
        pending = deque()       # (h, c, ex, ctxq_tile) awaiting pv
        ctxq_by_c = {}
        ct_by_qt = {}

        def pv_piece(ph, pc_, pex, qt_i, pc_t):
            cw = CW[pc_]
            qco = qt_i * 128
            qw = min(128, cw - qco)
            for kk in range(KS):
                sp = _sk(kk)
                nc.tensor.matmul(
                    pc_t[0:qw, qt_i, :],
                    lhsT=pex[0:sp, kk, qco:qco + qw],
                    rhs=v[0:sp, kk, ph, :],
                    start=(kk == 0), stop=(kk == KS - 1))

        def new_ct(qt):
            ct = ctpool.tile([128, MD, 128], dt.float16, name=f"ct{qt}",
                             tag="ctxT")
            ct_by_qt[qt] = ct
            return ct

        def pv_finish(ph, pc_, pcq, pc_t):
            last = ph == HPG - 1
            if last and pc_ == 2:
                # end-game: per-qtile recip+norm on DVE, then PE transposes
                # (fp16 PSUM) + ACT copies into per-qtile ctxT tiles.
                for qt_i in range(4):
                    zq = zpool.tile([128, 1, 1], dt.float32, name=f"zq{qt_i}",
                                    tag="zr")
                    nc.vector.reciprocal(zq[:], pc_t[:, qt_i:qt_i + 1,
                                                     DH:DH + 1])
                    nc.vector.tensor_tensor(
                        pcq[:, qt_i, ph * DH:(ph + 1) * DH],
                        pc_t[:, qt_i, 0:DH],
                        zq[:, 0].to_broadcast([128, DH]), op=ALU.mult)
                psts = []
                for qt_i in range(4):
                    pst = psc.tile([128, MD, 128], dt.float16,
                                   name=f"pst{qt_i}", tag="psc")
                    for m in range(MD):
                        nc.tensor.transpose(
                            pst[:, m, :],
                            pcq[0:128, qt_i, m * 128:(m + 1) * 128],
                            ident[:])
                    psts.append(pst)
                for qt_i in range(4):
                    qt = 4 * pc_ + qt_i
                    ct = new_ct(qt)
                    nc.scalar.copy(ct[:], psts[qt_i][:])
                    ct_cast(qt, fast=True)
                return
            zr = zpool.tile([128, 4, 1], dt.float32, tag="zr")
            nc.vector.reciprocal(zr[:], pc_t[:, :, DH:DH + 1])
            nc.vector.tensor_tensor(
                pcq[:, :, ph * DH:(ph + 1) * DH], pc_t[:, :, 0:DH],
                zr[:].to_broadcast([128, 4, DH]), op=ALU.mult)
            if last:
                for qt_i in range(4):
                    qt = 4 * pc_ + qt_i
                    pp = 128 if _sk(qt) == 128 else 96
                    ct = new_ct(qt)
                    nc.sync.dma_start_transpose(
                        out=ct[:, :, 0:pp],
                        in_=pcq[0:pp, qt_i, :])
                    pieces.appendleft(CC(qt))

        def get_ctxq(c):
            if c not in ctxq_by_c:
                ctxq_by_c[c] = cqpool.tile([128, 4, DG], dt.float16,
                                           name=f"ctxq{c}", tag="ctxq")
            return ctxq_by_c[c]

        def scores_pair(h, c, kk2, ex):
            base = 64 * (h % 2)
            td = h // 2
            cw, co = CW[c], CO[c]
            ps = ps2.tile([128, 2, 512], dt.float32, tag="ps2")
            for j in range(2):
                kk = kk2 + j
                sp = _sk(kk)
                nc.tensor.matmul(
                    ps[0:sp, j, 0:cw],
                    lhsT=kT[base:base + 64, td, kk * 128:kk * 128 + sp],
                    rhs=qT[base:base + 64, td, co:co + cw],
                    start=True, stop=True)
            nc.scalar.activation(ex[:, kk2:kk2 + 2, 0:cw], ps[:, :, 0:cw],
                                 AF.Exp)

        slot_no = [0]

        def unit(h, c, budget=2200):
            get_ctxq(c)
            m = h // 2
            prereq = [("k", m, 0), ("k", m, 1), ("k", m, 2), ("q", m, c)]
            lag = 3 if slot_no[0] < 5 else 2
            slot_no[0] += 1
            do_pv = len(pending) >= lag
            if do_pv:
                ph = pending[0][0]
                prereq += [("v", ph // 5, ms) for ms in range(KS)]
            drain_until(prereq)
            popped = None
            if do_pv:
                ph, pc_, pex, pcq = pending.popleft()
                popped = (ph, pc_)
                pc_t = pv_psum()
            ex = epool.tile([128, KS, 512], dt.float16, tag="expT")
            for kk2 in range(0, KS, 2):
                scores_pair(h, c, kk2, ex)
                if do_pv and kk2 >= 4:      # weave pv qtiles btwn pairs 3..6
                    pv_piece(ph, pc_, pex, kk2 // 2 - 2, pc_t)
            if do_pv:
                pv_finish(ph, pc_, pcq, pc_t)
            pending.append((h, c, ex, ctxq_by_c[c]))
            pop_budget(budget)
            return popped

        def flush():
            ph, pc_, pex, pcq = pending.popleft()
            drain_until([("v", ph // 5, ms) for ms in range(KS)])
            pc_t = pv_psum()
            for qt_i in range(4):
                if pieces:
                    pop_one()
                pv_piece(ph, pc_, pex, qt_i, pc_t)
            pv_finish(ph, pc_, pcq, pc_t)

        # ---- preamble: unit(0,0)'s score pairs hand-woven between the
        # projection chains they depend on, so ACT exp starts early.
        qk_chain("k", 0, 0, co=0, cw=128)
        qk_chain("q", 0, 0, co=0, cw=128)
        qk_chain("k", 0, 0, co=128, cw=384)
        qk_chain("q", 0, 0, co=128, cw=384)
        ex0 = epool.tile([128, KS, 512], dt.float16, tag="expT")
        scores_pair(0, 0, 0, ex0)
        scores_pair(0, 0, 2, ex0)
        v_chain(0, 0)
        v_chain(0, 1)
        qk_chain("k", 1, 0)
        qk_chain("q", 1, 0)
        qk_chain("k", 0, 1)
        scores_pair(0, 0, 4, ex0)
        scores_pair(0, 0, 6, ex0)
        v_chain(0, 2)
        v_chain(0, 3)
        qk_chain("k", 1, 1)
        qk_chain("k", 0, 2)
        scores_pair(0, 0, 8, ex0)
        scores_pair(0, 0, 10, ex0)
        pending.append((0, 0, ex0, get_ctxq(0)))
        emitted.update([("k", 1, 0), ("k", 1, 1), ("q", 1, 0)] +
                       [("v", 0, i) for i in range(4)])

        # piece FIFO in first-use order (see unit sequence below).
        pieces.extend([QK("q", 0, 1), QK("q", 1, 1), QK("k", 1, 2),
                       QK("q", 0, 2), QK("q", 1, 2)])
        pieces.extend(VC(0, ms) for ms in range(4, KS))
        pieces.extend([DMAW(("dma", "k2"), qk_dma, 2, "k"),
                       DMAW(("dma", "q2"), qk_dma, 2, "q"),
                       DMAW(("dma", "v1"), v_dma, 1),
                       QK("k", 2, 0), QK("k", 2, 1), QK("k", 2, 2),
                       QK("q", 2, 0), QK("q", 2, 1)])
        pieces.extend(VC(1, ms) for ms in range(KS))
        pieces.extend([DMAW(("dma", "k3"), qk_dma, 3, "k"),
                       DMAW(("dma", "q3"), qk_dma, 3, "q"),
                       QK("k", 3, 0), QK("k", 3, 1), QK("k", 3, 2),
                       QK("q", 3, 0), QK("q", 3, 1),
                       DMAW(("dma", "k4"), qk_dma, 4, "k"),
                       DMAW(("dma", "q4"), qk_dma, 4, "q"),
                       QK("k", 4, 0), QK("k", 4, 1), QK("k", 4, 2),
                       QK("q", 4, 0), QK("q", 4, 1),
                       DMAW(("dma", "wo"),
                            lambda: nc.sync.dma_start(out=wo_s[:],
                                                      in_=wo_d[:]))])
        pieces.extend([QK("q", m, 2) for m in range(2, MD)])

        # unit sequence: chunk-0/1 units interleaved, ordered so units on
        # already-projected kT planes run first -- ACT exp saturates early
        # while the remaining k-plane/v projections drain behind it.
        seq = [(1, 0), (0, 1), (1, 1), (0, 2), (1, 2)]
        for m in range(1, MD - 1):
            seq += [(2 * m, 0), (2 * m, 1), (2 * m + 1, 0), (2 * m + 1, 1)]
        seq += [(8, 0), (9, 0), (8, 1), (9, 1)]
        seq += [(h, 2) for h in range(2, HPG)]
        budgets = {0: 1700, 1: 1800, 2: 2000}
        lean = [True] * 8

        for h, c in seq:
            b = budgets[c]
            if lean:
                b = 1000
                lean.pop()
            popped = unit(h, c, budget=b)
            if popped == (HPG - 1, 0):
                for qt in range(0, 4):
                    pieces.extend(OP(qt, ni) for ni in range(3))
            elif popped == (HPG - 1, 1):
                for qt in range(4, 6):   # qt 6/7 are emitted inline at the
                    pieces.extend(OP(qt, ni) for ni in range(3))  # tail

        flush()                 # pv(7,2)
        for ni in range(3):     # reserved: rides out exp(8,2)/(9,2) latency
            oproj_chain(6, ni)
        flush()                 # pv(8,2)
        for ni in range(3):
            oproj_chain(7, ni)
        # last flush, fully pipelined per qtile: pv(qt) -> recip/norm (DVE)
        # -> PE transpose of qt-1 between pv pieces -> ACT copy + fp8 cast,
        # so the final o-proj's first dependencies land while pv still runs.
        lh, lc, lex, lcq = pending.popleft()
        lpc = pv_psum()
        for qt_i in range(4):
            pv_piece(lh, lc, lex, qt_i, lpc)
            zq = zpool.tile([128, 1, 1], dt.float32, name=f"lzq{qt_i}",
                            tag="zr")
            nc.vector.reciprocal(zq[:], lpc[:, qt_i:qt_i + 1, DH:DH + 1])
            nc.vector.tensor_tensor(
                lcq[:, qt_i, lh * DH:(lh + 1) * DH],
                lpc[:, qt_i, 0:DH],
                zq[:, 0].to_broadcast([128, DH]), op=ALU.mult)
            if qt_i >= 1:
                pst = psc.tile([128, MD, 128], dt.float16,
                               name=f"lpst{qt_i - 1}", tag="psc")
                for m in range(MD):
                    nc.tensor.transpose(
                        pst[:, m, :],
                        lcq[0:128, qt_i - 1, m * 128:(m + 1) * 128],
                        ident[:])
                qt = 8 + qt_i - 1
                ct = new_ct(qt)
                nc.scalar.copy(ct[:], pst[:])
                ct_cast(qt, fast=True)
                oproj_chain(qt, 0, act_copy=True)
        pst = psc.tile([128, MD, 128], dt.float16, name="lpst3", tag="psc")
        for m in range(MD):
            nc.tensor.transpose(
                pst[:, m, :], lcq[0:128, 3, m * 128:(m + 1) * 128], ident[:])
        ct = new_ct(11)
        nc.scalar.copy(ct[:], pst[:])
        ct_cast(11, fast=True)
        pop_budget(10**9)       # anything left
        for ni in (1, 2):
            for qt in range(8, 12):
                oproj_chain(qt, ni, act_copy=True,
                            dma_act=(qt % 2 == 1 and ni < 2))
        oproj_chain(11, 0, act_copy=True)

    nc.compile()
    return nc


def _get_nc():
    if "nc" not in _CACHE:
        _CACHE["nc"] = build()
    return _CACHE["nc"]


def _split8(a):
    hi = a.astype(E4)
    lo = (a - hi.astype(np.float32)).astype(E4)
    return hi, lo


def _prep_in_maps(x, Wq, bq, Wk, Wv, Wo):
    in_maps = []
    for c in range(N_CORES):
        b, g = divmod(c, G)
        gs = slice(g * DG, (g + 1) * DG)
        # x planes: [kk, part, S] -> [part, KP, 2, S]
        xT = np.ascontiguousarray(x[b].T).astype(np.float32)
        xp = xT.reshape(KP, 2, 128, S).transpose(2, 0, 1, 3)
        xpad = np.zeros((128, KP, 2, SP_), dtype=np.float32)
        xpad[:, :, :, :S] = xp
        x8, dx = _split8(xpad)

        def wqk_prep(W):
            w = (WSCALE * W[gs, :].T).astype(np.float32)      # [D, DG]
            w = w.reshape(KD, 128, MD, 128)                   # [kk, part, m, col]
            w8, wd = _split8(w)
            # -> [MD, part, which, KP, 2, 128]
            def lay(a):
                return a.reshape(KP, 2, 128, MD, 128).transpose(3, 2, 0, 1, 4)
            return np.ascontiguousarray(
                np.stack([lay(w8), lay(wd)], axis=2))

        def wv_prep(W):
            w = (WSCALE * W[gs, :].T).astype(np.float32)
            w = w.reshape(KD, 128, G, 320)                    # [kk, part, n, col]
            w8, wd = _split8(w)
            def lay(a):
                return a.reshape(KP, 2, 128, G, 320).transpose(3, 2, 0, 1, 4)
            return np.ascontiguousarray(np.stack([lay(w8), lay(wd)], axis=2))

        # o-proj: [plane, part, col] blocks
        wo_ = (WSCALE * Wo[:, gs].T).astype(np.float32).reshape(MD, 128, D)
        wo8, wod = _split8(wo_)
        blocks = [
            np.stack([wo8[0], wo8[1]], axis=1),
            np.stack([wo8[2], wo8[3]], axis=1),
            np.stack([wod[0], wod[1]], axis=1),
            np.stack([wod[2], wod[3]], axis=1),
            np.stack([wo8[4], wo8[4]], axis=1),
            np.stack([wod[4], wod[4]], axis=1),
        ]
        wo_t = np.ascontiguousarray(
            np.stack(blocks, axis=0).transpose(1, 0, 2, 3))   # [128,6,2,D]

        in_maps.append({
            "x8": np.ascontiguousarray(x8),
            "dx": np.ascontiguousarray(dx),
            "wq": wqk_prep(Wq),
            "wk": wqk_prep(Wk),
            "wv": wv_prep(Wv),
            "wo": wo_t,
            "bq": np.ascontiguousarray(
                (0.125 * bq[gs]).astype(np.float32).reshape(MD, 128).T),
        })
    return in_maps


def run(x, Wq, bq, Wk, Wv, bv, Wo, bo, trace=False, **trace_kw):
    x = np.asarray(x, dtype=np.float32)
    Wq = np.asarray(Wq, dtype=np.float32)
    bq = np.asarray(bq, dtype=np.float32)
    Wk = np.asarray(Wk, dtype=np.float32)
    Wv = np.asarray(Wv, dtype=np.float32)
    bv = np.asarray(bv, dtype=np.float32)
    Wo = np.asarray(Wo, dtype=np.float32)
    bo = np.asarray(bo, dtype=np.float32)

    nc = _get_nc()
    in_maps = _prep_in_maps(x, Wq, bq, Wk, Wv, Wo)
    res = None
    for attempt in range(3):
        try:
            res = run_bass_kernel_spmd(nc, in_maps, list(range(N_CORES)),
                                       trace=trace, **trace_kw)
            break
        except Exception:
            # Sporadic NRT_EXEC_UNIT_UNRECOVERABLE on first exec; devices
            # come back after ~75s. Reset the backend and retry.
            if attempt == 2:
                raise
            import time as _time
            import jax as _jax
            _time.sleep(80)
            try:
                _jax.clear_backends()
            except Exception:
                pass
    const = (bv @ Wo.T + bo).astype(np.float32)  # [D]
    out = np.empty((B, S, D), dtype=np.float32)
    for b in range(B):
        out[b] = res.results[2 * b]["out"] + res.results[2 * b + 1]["out"] + const
    return out, res


def kernel(**inputs):
    out, _ = run(**inputs)
    return out


# revision 30
# speedup vs baseline: 1.0816x; 1.0040x over previous
"""Trainium2 Bass kernel: Whisper-style self-attention (B=4, S=1500, D=1280, H=20).

Sharding: core c = 2*b + g handles batch b (of 4) and head-group g (of 2,
10 heads each).  Q/K/V projections column-sharded over the head group,
attention sharded by (batch, head), output projection row-sharded; the two
head-group partials of each batch are summed on the host (plus bias terms).

v3 dataflow: projections run as fp8e4 DoubleRow matmuls (0.5 cycles/row,
2 contraction planes per instruction) with 3-term error compensation:
x ~ x8+dx, W ~ W8+dW (all e4m3, W pre-scaled x32 so residuals clear the
subnormal floor), computing x8W8 + x8dW + dxW8 -- ~fp16 accuracy at 0.75x
the fp16 PE cost for q/k/v and 0.8x for the o-projection (ctx split into
c8+dc on the gpsimd engine after the f16 transpose).  Scale bookkeeping:
q drain mult 0.125/32 (+0.125bq), k/v drains mult 1/32, PV "ones" column
1/16 (so ctxq = 16*ctx, putting the fp8 ctx split in range), o-proj drain
mult 1/512.  Scores/softmax/PV stay fp16 (fp8 attention weights flush to
zero below e4m3's subnormal floor and crater accuracy).

Scheduling: units are woven as [score-pair, pv-piece, score-pair, ...] with
the pv of unit u-2 riding between unit u's score pairs, so the PE never
waits on ACT exp draining the 2-buf score PSUM.  Projection / O-proj work
sits in a prerequisite-keyed FIFO of ~1-2us pieces; each unit first drains
the pieces its scores/pv depend on, then pops a tunable extra budget.
"""
import sys
sys.path.insert(0, "/opt/trn_rl_repo")

from collections import deque
from contextlib import ExitStack
import numpy as np
import ml_dtypes

import concourse.bass as bass
import concourse.tile as tile
from concourse import bacc, mybir
from concourse.bass_utils import run_bass_kernel_spmd

dt = mybir.dt
AF = mybir.ActivationFunctionType
ALU = mybir.AluOpType
PM = mybir.MatmulPerfMode
E4 = ml_dtypes.float8_e4m3

N_CORES = 8
B, S, D = 4, 1500, 1280
H, DH = 20, 64
G = 2
DG = D // G            # 640
HPG = H // G           # 10
KD = D // 128          # 10 contraction planes for D
KP = KD // 2           # 5 DoubleRow plane-pairs
MD = DG // 128         # 5 dh-planes per group
CW = (512, 512, 476)   # q/proj chunk widths
CO = (0, 512, 1024)
NS = 3
KS = (S + 127) // 128  # 12 k-tiles (11*128 + 92)
SP_ = 12 * 128         # 1536: padded S for ctxT columns
ON = (512, 512, 256)   # o-proj n chunks
WSCALE = 32.0          # fp8 weight pre-scale (subnormal headroom)
CSCALE = 16.0          # ctx pre-scale for fp8 split

_CACHE = {}


def _sk(i):
    return min(128, S - i * 128)


def build():
    nc = bacc.Bacc("TRN2", target_bir_lowering=False, debug=False,
                   num_devices=N_CORES)
    x8_d = nc.dram_tensor("x8", [128, KP, 2, SP_], dt.float8e4,
                          kind="ExternalInput").ap()
    dx_d = nc.dram_tensor("dx", [128, KP, 2, SP_], dt.float8e4,
                          kind="ExternalInput").ap()
    # [m, part, which(W8/dW), pair, parity, col]
    wq_d = nc.dram_tensor("wq", [MD, 128, 2, KP, 2, 128], dt.float8e4,
                          kind="ExternalInput").ap()
    wk_d = nc.dram_tensor("wk", [MD, 128, 2, KP, 2, 128], dt.float8e4,
                          kind="ExternalInput").ap()
    wv_d = nc.dram_tensor("wv", [G, 128, 2, KP, 2, 320], dt.float8e4,
                          kind="ExternalInput").ap()
    # o-proj rhs blocks: A=(W8p0,W8p1) B=(W8p2,W8p3) C=(dWp0,dWp1)
    # D=(dWp2,dWp3) E=(W8p4,W8p4) F=(dWp4,dWp4)
    wo_d = nc.dram_tensor("wo", [128, 6, 2, D], dt.float8e4,
                          kind="ExternalInput").ap()
    bq_d = nc.dram_tensor("bq", [128, MD], dt.float32, kind="ExternalInput").ap()
    out_d = nc.dram_tensor("out", [S, D], dt.float32, kind="ExternalOutput").ap()

    with tile.TileContext(nc) as tc, ExitStack() as octx:
        persist = octx.enter_context(tc.tile_pool(name="persist", bufs=1))
        wqkp = octx.enter_context(tc.tile_pool(name="wqk", bufs=6))
        wvp = octx.enter_context(tc.tile_pool(name="wv", bufs=2))
        epool = octx.enter_context(tc.tile_pool(name="expT", bufs=4))
        cqpool = octx.enter_context(tc.tile_pool(name="ctxq", bufs=3))
        zpool = octx.enter_context(tc.tile_pool(name="z", bufs=3))
        opool = octx.enter_context(tc.tile_pool(name="ob", bufs=3))
        ctpool = octx.enter_context(tc.tile_pool(name="ctxT", bufs=4))
        ps1 = octx.enter_context(tc.tile_pool(name="ps1", bufs=2, space="PSUM"))
        ps2 = octx.enter_context(tc.tile_pool(name="ps2", bufs=2, space="PSUM"))
        psc = octx.enter_context(tc.tile_pool(name="psc", bufs=2, space="PSUM"))

        def pv_psum():
            return psc.tile([128, 4, DH + 1], dt.float32,
                            name="pvps", tag="psc")

        x8_s = persist.tile([128, KP, 2, SP_], dt.float8e4, tag="x8")
        dx_s = persist.tile([128, KP, 2, SP_], dt.float8e4, tag="dx")
        qT = persist.tile([128, MD, S], dt.float16, tag="qT")
        kT = persist.tile([128, MD, S], dt.float16, tag="kT")
        v = persist.tile([128, KS, HPG, DH + 1], dt.float16, tag="v")
        cT2 = persist.tile([128, MD, 2, SP_], dt.float8e4, tag="cT2")
        wo_s = persist.tile([128, 6, 2, D], dt.float8e4, tag="wo")
        bq_s = persist.tile([128, MD], dt.float32, tag="bq")

        wts = {}

        def qk_dma(m, which):
            w_d = wq_d if which == "q" else wk_d
            wt = wqkp.tile([128, 2, KP, 2, 128], dt.float8e4,
                           name=f"w{which}{m}", tag="wqk")
            nc.sync.dma_start(out=wt[:], in_=w_d[m])
            wts[which, m] = wt

        def v_dma(n):
            wt = wvp.tile([128, 2, KP, 2, 320], dt.float8e4, name=f"wv{n}",
                          tag="wv")
            nc.sync.dma_start(out=wt[:], in_=wv_d[n])
            wts["v", n] = wt

        # startup DMA order is tuned so the PE preamble below never waits.
        qk_dma(0, "k")
        nc.sync.dma_start(out=x8_s[:, :, :, 0:512], in_=x8_d[:, :, :, 0:512])
        nc.sync.dma_start(out=dx_s[:, :, :, 0:512], in_=dx_d[:, :, :, 0:512])
        qk_dma(0, "q")
        nc.sync.dma_start(out=bq_s[:], in_=bq_d[:])
        v_dma(0)
        qk_dma(1, "k")
        qk_dma(1, "q")
        for co in range(512, S, 512):
            cw = min(512, S - co)
            nc.sync.dma_start(out=x8_s[:, :, :, co:co + cw],
                              in_=x8_d[:, :, :, co:co + cw])
            nc.sync.dma_start(out=dx_s[:, :, :, co:co + cw],
                              in_=dx_d[:, :, :, co:co + cw])

        from concourse.masks import make_identity
        ident = persist.tile([128, 128], dt.float16, tag="ident")
        make_identity(nc, ident[:])
        ones1 = persist.tile([128, 1], dt.float16, tag="ones1")
        nc.vector.memset(ones1[:], 1.0 / CSCALE)
        nc.vector.tensor_copy(v[:, :, :, DH:DH + 1],
                              ones1[:].to_broadcast([128, KS, HPG, 1]))

        # DoubleRow 3-term order: (x-src, w-sel) per plane-pair
        TERMS = ((0, 0), (1, 0), (0, 1))   # x8*W8, dx*W8, x8*dW

        def qk_chain(which, m, n, co=None, cw=None):
            """One projection chain: qT/kT plane m, column chunk n. ~1.6us."""
            wt = wts[which, m]
            dst = qT if which == "q" else kT
            if co is None:
                cw, co = CW[n], CO[n]
            ps = ps1.tile([128, 1, 512], dt.float32, tag="ps1")
            n_inst = len(TERMS) * KP
            i = 0
            for xsel, wsel in TERMS:
                xsrc = x8_s if xsel == 0 else dx_s
                for pp in range(KP):
                    nc.tensor.matmul(
                        ps[:, 0, 0:cw],
                        lhsT=wt[:, wsel, pp],
                        rhs=xsrc[:, pp, :, co:co + cw],
                        perf_mode=PM.DoubleRow,
                        start=(i == 0), stop=(i == n_inst - 1))
                    i += 1
            osl = dst[:, m, co:co + cw]
            if which == "q":
                nc.vector.tensor_scalar(
                    osl, ps[:, 0, 0:cw], 0.125 / WSCALE, bq_s[:, m:m + 1],
                    op0=ALU.mult, op1=ALU.add)
            else:
                nc.vector.tensor_scalar_mul(osl, ps[:, 0, 0:cw], 1.0 / WSCALE)

        def v_chain(n, ms):
            """v columns for heads 5n..5n+4, s-tile ms. ~1.0us."""
            wt = wts["v", n]
            sp = _sk(ms)
            ps = ps1.tile([128, 1, 512], dt.float32, tag="ps1")
            n_inst = len(TERMS) * KP
            i = 0
            for xsel, wsel in TERMS:
                xsrc = x8_s if xsel == 0 else dx_s
                for pp in range(KP):
                    nc.tensor.matmul(
                        ps[0:sp, 0, 0:320],
                        lhsT=xsrc[:, pp, :, ms * 128:ms * 128 + sp],
                        rhs=wt[:, wsel, pp],
                        perf_mode=PM.DoubleRow,
                        start=(i == 0), stop=(i == n_inst - 1))
                    i += 1
            nc.vector.tensor_scalar_mul(
                v[0:sp, ms, n * 5:(n + 1) * 5, 0:DH],
                ps[0:sp, 0, 0:320].rearrange("p (h e) -> p h e", h=5),
                1.0 / WSCALE)

        def ct_cast(qt, fast=False):
            """ctxT f16 [128, MD, 128] for q-tile qt -> cT2 fp8 (c8, dc).
            The two passes split across engines so consecutive q-tiles
            pipeline instead of serializing on gpsimd."""
            ct = ct_by_qt.pop(qt)
            qo = qt * 128
            ceng, seng = (nc.vector, nc.gpsimd) if fast else (nc.gpsimd,
                                                              nc.vector)
            ceng.tensor_copy(cT2[:, :, 0, qo:qo + 128], ct[:])
            seng.tensor_tensor(
                cT2[:, :, 1, qo:qo + 128], ct[:],
                cT2[:, :, 0, qo:qo + 128], op=ALU.subtract)

        def oproj_chain(qt, ni, act_copy=False, dma_act=False):
            """One o-proj chain: q-tile qt, n-chunk ni (8 DoubleRow insts).
            act_copy routes the PSUM drain to the ACT engine (idle at the
            kernel tail) so it can't delay DVE's critical recip/norm."""
            sp = _sk(qt)
            mw = 128 if sp == 128 else 96
            nw = ON[ni]
            noff = CO[ni]
            qo = qt * 128
            ps = ps1.tile([128, 1, 512], dt.float32, tag="ps1")
            insts = [
                (cT2[:, 0:2, 0, qo:qo + mw], 0),
                (cT2[:, 2:4, 0, qo:qo + mw], 1),
                (cT2[:, 0:2, 1, qo:qo + mw], 0),
                (cT2[:, 2:4, 1, qo:qo + mw], 1),
                (cT2[:, 0:2, 0, qo:qo + mw], 2),
                (cT2[:, 2:4, 0, qo:qo + mw], 3),
                (cT2[:, 4, 0:2, qo:qo + mw], 4),
                (cT2[:, 4, 0:2, qo:qo + mw], 5),
            ]
            for i, (lh, wb) in enumerate(insts):
                nc.tensor.matmul(
                    ps[0:mw, 0, 0:nw],
                    lhsT=lh,
                    rhs=wo_s[:, wb, :, noff:noff + nw],
                    perf_mode=PM.DoubleRow,
                    start=(i == 0), stop=(i == len(insts) - 1))
            ob = opool.tile([128, 512], dt.float32, tag="ob")
            if act_copy:
                nc.scalar.mul(ob[0:sp, 0:nw], ps[0:sp, 0, 0:nw],
                              1.0 / (WSCALE * CSCALE))
            else:
                nc.vector.tensor_scalar_mul(ob[0:sp, 0:nw], ps[0:sp, 0, 0:nw],
                                            1.0 / (WSCALE * CSCALE))
            deng = nc.scalar if dma_act else nc.sync
            deng.dma_start(
                out=out_d[qt * 128:qt * 128 + sp, noff:noff + nw],
                in_=ob[0:sp, 0:nw])

        # ---- prerequisite-keyed filler piece queue --------------------
        pieces = deque()        # (key, fn, cost_ns)
        emitted = set()
        emitted.update([("k", 0, 0), ("k", 0, 1), ("k", 0, 2), ("q", 0, 0)])

        def pop_one():
            key, fn, cost = pieces.popleft()
            fn()
            emitted.add(key)
            return cost

        def drain_until(keys):
            need = [k for k in keys if k not in emitted]
            for k in need:
                while k not in emitted:
                    assert pieces, f"piece schedule missing prerequisite {k}"
                    pop_one()

        def pop_budget(budget):
            while budget > 0 and pieces:
                budget -= pop_one()

        def QK(which, m, n):
            return ((which, m, n), lambda: qk_chain(which, m, n), 1650)

        def VC(n, ms):
            return (("v", n, ms), lambda: v_chain(n, ms), 1050)

        def DMAW(key, fn, *a):
            return (key, lambda: fn(*a), 50)

        def OP(qt, ni):
            return (("op", qt, ni), lambda: oproj_chain(qt, ni), 900)

        def CC(qt):
            return (("cc", qt), lambda: ct_cast(qt), 100)

        # ---- attention unit, woven ------------------------------------
        pending = deque()       # (h, c, ex, ctxq_tile) awaiting pv
        ctxq_by_c = {}
        ct_by_qt = {}

        def pv_piece(ph, pc_, pex, qt_i, pc_t):
            cw = CW[pc_]
            qco = qt_i * 128
            qw = min(128, cw - qco)
            for kk in range(KS):
                sp = _sk(kk)
                nc.tensor.matmul(
                    pc_t[0:qw, qt_i, :],
                    lhsT=pex[0:sp, kk, qco:qco + qw],
                    rhs=v[0:sp, kk, ph, :],
                    start=(kk == 0), stop=(kk == KS - 1))

        def new_ct(qt):
            ct = ctpool.tile([128, MD, 128], dt.float16, name=f"ct{qt}",
                             tag="ctxT")
            ct_by_qt[qt] = ct
            return ct

        def pv_finish(ph, pc_, pcq, pc_t):
            last = ph == HPG - 1
            if last and pc_ == 2:
                # end-game: per-qtile recip+norm on DVE, then PE transposes
                # (fp16 PSUM) + ACT copies into per-qtile ctxT tiles.
                for qt_i in range(4):
                    zq = zpool.tile([128, 1, 1], dt.float32, name=f"zq{qt_i}",
                                    tag="zr")
                    nc.vector.reciprocal(zq[:], pc_t[:, qt_i:qt_i + 1,
                                                     DH:DH + 1])
                    nc.vector.tensor_tensor(
                        pcq[:, qt_i, ph * DH:(ph + 1) * DH],
                        pc_t[:, qt_i, 0:DH],
                        zq[:, 0].to_broadcast([128, DH]), op=ALU.mult)
                psts = []
                for qt_i in range(4):
                    pst = psc.tile([128, MD, 128], dt.float16,
                                   name=f"pst{qt_i}", tag="psc")
                    for m in range(MD):
                        nc.tensor.transpose(
                            pst[:, m, :],
                            pcq[0:128, qt_i, m * 128:(m + 1) * 128],
                            ident[:])
                    psts.append(pst)
                for qt_i in range(4):
                    qt = 4 * pc_ + qt_i
                    ct = new_ct(qt)
                    nc.scalar.copy(ct[:], psts[qt_i][:])
                    ct_cast(qt, fast=True)
                return
            zr = zpool.tile([128, 4, 1], dt.float32, tag="zr")
            nc.vector.reciprocal(zr[:], pc_t[:, :, DH:DH + 1])
            nc.vector.tensor_tensor(
                pcq[:, :, ph * DH:(ph + 1) * DH], pc_t[:, :, 0:DH],
                zr[:].to_broadcast([128, 4, DH]), op=ALU.mult)
            if last:
                for qt_i in range(4):
                    qt = 4 * pc_ + qt_i
                    pp = 128 if _sk(qt) == 128 else 96
                    ct = new_ct(qt)
                    nc.sync.dma_start_transpose(
                        out=ct[:, :, 0:pp],
                        in_=pcq[0:pp, qt_i, :])
                    pieces.appendleft(CC(qt))

        def get_ctxq(c):
            if c not in ctxq_by_c:
                ctxq_by_c[c] = cqpool.tile([128, 4, DG], dt.float16,
                                           name=f"ctxq{c}", tag="ctxq")
            return ctxq_by_c[c]

        def scores_pair(h, c, kk2, ex):
            base = 64 * (h % 2)
            td = h // 2
            cw, co = CW[c], CO[c]
            ps = ps2.tile([128, 2, 512], dt.float32, tag="ps2")
            for j in range(2):
                kk = kk2 + j
                sp = _sk(kk)
                nc.tensor.matmul(
                    ps[0:sp, j, 0:cw],
                    lhsT=kT[base:base + 64, td, kk * 128:kk * 128 + sp],
                    rhs=qT[base:base + 64, td, co:co + cw],
                    start=True, stop=True)
            nc.scalar.activation(ex[:, kk2:kk2 + 2, 0:cw], ps[:, :, 0:cw],
                                 AF.Exp)

        slot_no = [0]

        def unit(h, c, budget=2200):
            get_ctxq(c)
            m = h // 2
            prereq = [("k", m, 0), ("k", m, 1), ("k", m, 2), ("q", m, c)]
            lag = 3 if slot_no[0] < 5 else 2
            slot_no[0] += 1
            do_pv = len(pending) >= lag
            if do_pv:
                ph = pending[0][0]
                prereq += [("v", ph // 5, ms) for ms in range(KS)]
            drain_until(prereq)
            popped = None
            if do_pv:
                ph, pc_, pex, pcq = pending.popleft()
                popped = (ph, pc_)
                pc_t = pv_psum()
            ex = epool.tile([128, KS, 512], dt.float16, tag="expT")
            for kk2 in range(0, KS, 2):
                scores_pair(h, c, kk2, ex)
                if do_pv and kk2 >= 4:      # weave pv qtiles btwn pairs 3..6
                    pv_piece(ph, pc_, pex, kk2 // 2 - 2, pc_t)
            if do_pv:
                pv_finish(ph, pc_, pcq, pc_t)
            pending.append((h, c, ex, ctxq_by_c[c]))
            pop_budget(budget)
            return popped

        def flush():
            ph, pc_, pex, pcq = pending.popleft()
            drain_until([("v", ph // 5, ms) for ms in range(KS)])
            pc_t = pv_psum()
            for qt_i in range(4):
                if pieces:
                    pop_one()
                pv_piece(ph, pc_, pex, qt_i, pc_t)
            pv_finish(ph, pc_, pcq, pc_t)

        # ---- preamble: unit(0,0)'s score pairs hand-woven between the
        # projection chains they depend on, so ACT exp starts early.
        qk_chain("k", 0, 0, co=0, cw=128)
        qk_chain("q", 0, 0, co=0, cw=128)
        qk_chain("k", 0, 0, co=128, cw=384)
        qk_chain("q", 0, 0, co=128, cw=384)
        ex0 = epool.tile([128, KS, 512], dt.float16, tag="expT")
        scores_pair(0, 0, 0, ex0)
        scores_pair(0, 0, 2, ex0)
        v_chain(0, 0)
        v_chain(0, 1)
        qk_chain("k", 1, 0)
        qk_chain("q", 1, 0)
        qk_chain("k", 0, 1)
        scores_pair(0, 0, 4, ex0)
        scores_pair(0, 0, 6, ex0)
        v_chain(0, 2)
        v_chain(0, 3)
        qk_chain("k", 1, 1)
        qk_chain("k", 0, 2)
        scores_pair(0, 0, 8, ex0)
        scores_pair(0, 0, 10, ex0)
        pending.append((0, 0, ex0, get_ctxq(0)))
        emitted.update([("k", 1, 0), ("k", 1, 1), ("q", 1, 0)] +
                       [("v", 0, i) for i in range(4)])

        # piece FIFO in first-use order (see unit sequence below).
        pieces.extend([QK("q", 0, 1), QK("q", 1, 1), QK("k", 1, 2),
                       QK("q", 0, 2), QK("q", 1, 2)])
        pieces.extend(VC(0, ms) for ms in range(4, KS))
        pieces.extend([DMAW(("dma", "k2"), qk_dma, 2, "k"),
                       DMAW(("dma", "q2"), qk_dma, 2, "q"),
                       DMAW(("dma", "v1"), v_dma, 1),
                       QK("k", 2, 0), QK("k", 2, 1), QK("k", 2, 2),
                       QK("q", 2, 0), QK("q", 2, 1)])
        pieces.extend(VC(1, ms) for ms in range(KS))
        pieces.extend([DMAW(("dma", "k3"), qk_dma, 3, "k"),
                       DMAW(("dma", "q3"), qk_dma, 3, "q"),
                       QK("k", 3, 0), QK("k", 3, 1), QK("k", 3, 2),
                       QK("q", 3, 0), QK("q", 3, 1),
                       DMAW(("dma", "k4"), qk_dma, 4, "k"),
                       DMAW(("dma", "q4"), qk_dma, 4, "q"),
                       QK("k", 4, 0), QK("k", 4, 1), QK("k", 4, 2),
                       QK("q", 4, 0), QK("q", 4, 1),
                       DMAW(("dma", "wo"),
                            lambda: nc.sync.dma_start(out=wo_s[:],
                                                      in_=wo_d[:]))])
        pieces.extend([QK("q", m, 2) for m in range(2, MD)])

        # unit sequence: chunk-0/1 units interleaved, ordered so units on
        # already-projected kT planes run first -- ACT exp saturates early
        # while the remaining k-plane/v projections drain behind it.
        seq = [(1, 0), (0, 1), (1, 1), (0, 2), (1, 2)]
        for m in range(1, MD - 1):
            seq += [(2 * m, 0), (2 * m, 1), (2 * m + 1, 0), (2 * m + 1, 1)]
        seq += [(8, 0), (9, 0), (8, 1), (9, 1)]
        seq += [(h, 2) for h in range(2, HPG)]
        budgets = {0: 1600, 1: 1700, 2: 1900}
        lean = [True] * 8

        for h, c in seq:
            b = budgets[c]
            if lean:
                b = 1000
                lean.pop()
            popped = unit(h, c, budget=b)
            if popped == (HPG - 1, 0):
                for qt in range(0, 4):
                    pieces.extend(OP(qt, ni) for ni in range(3))
            elif popped == (HPG - 1, 1):
                for qt in range(4, 6):   # qt 6/7 are emitted inline at the
                    pieces.extend(OP(qt, ni) for ni in range(3))  # tail

        flush()                 # pv(7,2)
        for ni in range(3):     # reserved: rides out exp(8,2)/(9,2) latency
            oproj_chain(6, ni)
        flush()                 # pv(8,2)
        for ni in range(3):
            oproj_chain(7, ni)
        # last flush, fully pipelined per qtile: pv(qt) -> recip/norm (DVE)
        # -> PE transpose of qt-1 between pv pieces -> ACT copy + fp8 cast,
        # so the final o-proj's first dependencies land while pv still runs.
        lh, lc, lex, lcq = pending.popleft()
        lpc = pv_psum()
        for qt_i in range(4):
            pv_piece(lh, lc, lex, qt_i, lpc)
            zq = zpool.tile([128, 1, 1], dt.float32, name=f"lzq{qt_i}",
                            tag="zr")
            nc.vector.reciprocal(zq[:], lpc[:, qt_i:qt_i + 1, DH:DH + 1])
            nc.vector.tensor_tensor(
                lcq[:, qt_i, lh * DH:(lh + 1) * DH],
                lpc[:, qt_i, 0:DH],
                zq[:, 0].to_broadcast([128, DH]), op=ALU.mult)
            if qt_i >= 1:
                pst = psc.tile([128, MD, 128], dt.float16,
                               name=f"lpst{qt_i - 1}", tag="psc")
                for m in range(MD):
                    nc.tensor.transpose(
                        pst[:, m, :],
                        lcq[0:128, qt_i - 1, m * 128:(m + 1) * 128],
                        ident[:])
                qt = 8 + qt_i - 1
                ct = new_ct(qt)
                nc.scalar.copy(ct[:], pst[:])
                ct_cast(qt, fast=True)
                oproj_chain(qt, 0, act_copy=True)
        pst = psc.tile([128, MD, 128], dt.float16, name="lpst3", tag="psc")
        for m in range(MD):
            nc.tensor.transpose(
                pst[:, m, :], lcq[0:128, 3, m * 128:(m + 1) * 128], ident[:])
        ct = new_ct(11)
        nc.scalar.copy(ct[:], pst[:])
        ct_cast(11, fast=True)
        pop_budget(10**9)       # anything left
        for ni in (1, 2):
            for qt in range(8, 12):
                oproj_chain(qt, ni, act_copy=True,
                            dma_act=(qt % 2 == 1 and ni < 2))
        oproj_chain(11, 0, act_copy=True)

    nc.compile()
    return nc


def _get_nc():
    if "nc" not in _CACHE:
        _CACHE["nc"] = build()
    return _CACHE["nc"]


def _split8(a):
    hi = a.astype(E4)
    lo = (a - hi.astype(np.float32)).astype(E4)
    return hi, lo


def _prep_in_maps(x, Wq, bq, Wk, Wv, Wo):
    in_maps = []
    for c in range(N_CORES):
        b, g = divmod(c, G)
        gs = slice(g * DG, (g + 1) * DG)
        # x planes: [kk, part, S] -> [part, KP, 2, S]
        xT = np.ascontiguousarray(x[b].T).astype(np.float32)
        xp = xT.reshape(KP, 2, 128, S).transpose(2, 0, 1, 3)
        xpad = np.zeros((128, KP, 2, SP_), dtype=np.float32)
        xpad[:, :, :, :S] = xp
        x8, dx = _split8(xpad)

        def wqk_prep(W):
            w = (WSCALE * W[gs, :].T).astype(np.float32)      # [D, DG]
            w = w.reshape(KD, 128, MD, 128)                   # [kk, part, m, col]
            w8, wd = _split8(w)
            # -> [MD, part, which, KP, 2, 128]
            def lay(a):
                return a.reshape(KP, 2, 128, MD, 128).transpose(3, 2, 0, 1, 4)
            return np.ascontiguousarray(
                np.stack([lay(w8), lay(wd)], axis=2))

        def wv_prep(W):
            w = (WSCALE * W[gs, :].T).astype(np.float32)
            w = w.reshape(KD, 128, G, 320)                    # [kk, part, n, col]
            w8, wd = _split8(w)
            def lay(a):
                return a.reshape(KP, 2, 128, G, 320).transpose(3, 2, 0, 1, 4)
            return np.ascontiguousarray(np.stack([lay(w8), lay(wd)], axis=2))

        # o-proj: [plane, part, col] blocks
        wo_ = (WSCALE * Wo[:, gs].T).astype(np.float32).reshape(MD, 128, D)
        wo8, wod = _split8(wo_)
        blocks = [
            np.stack([wo8[0], wo8[1]], axis=1),
            np.stack([wo8[2], wo8[3]], axis=1),
            np.stack([wod[0], wod[1]], axis=1),
            np.stack([wod[2], wod[3]], axis=1),
            np.stack([wo8[4], wo8[4]], axis=1),
            np.stack([wod[4], wod[4]], axis=1),
        ]
        wo_t = np.ascontiguousarray(
            np.stack(blocks, axis=0).transpose(1, 0, 2, 3))   # [128,6,2,D]

        in_maps.append({
            "x8": np.ascontiguousarray(x8),
            "dx": np.ascontiguousarray(dx),
            "wq": wqk_prep(Wq),
            "wk": wqk_prep(Wk),
            "wv": wv_prep(Wv),
            "wo": wo_t,
            "bq": np.ascontiguousarray(
                (0.125 * bq[gs]).astype(np.float32).reshape(MD, 128).T),
        })
    return in_maps


def run(x, Wq, bq, Wk, Wv, bv, Wo, bo, trace=False, **trace_kw):
    x = np.asarray(x, dtype=np.float32)
    Wq = np.asarray(Wq, dtype=np.float32)
    bq = np.asarray(bq, dtype=np.float32)
    Wk = np.asarray(Wk, dtype=np.float32)
    Wv = np.asarray(Wv, dtype=np.float32)
    bv = np.asarray(bv, dtype=np.float32)
    Wo = np.asarray(Wo, dtype=np.float32)
    bo = np.asarray(bo, dtype=np.float32)

    nc = _get_nc()
    in_maps = _prep_in_maps(x, Wq, bq, Wk, Wv, Wo)
    res = None
    for attempt in range(3):
        try:
            res = run_bass_kernel_spmd(nc, in_maps, list(range(N_CORES)),
                                       trace=trace, **trace_kw)
            break
        except Exception:
            # Sporadic NRT_EXEC_UNIT_UNRECOVERABLE on first exec; devices
            # come back after ~75s. Reset the backend and retry.
            if attempt == 2:
                raise
            import time as _time
            import jax as _jax
            _time.sleep(80)
            try:
                _jax.clear_backends()
            except Exception:
                pass
    const = (bv @ Wo.T + bo).astype(np.float32)  # [D]
    out = np.empty((B, S, D), dtype=np.float32)
    for b in range(B):
        out[b] = res.results[2 * b]["out"] + res.results[2 * b + 1]["out"] + const
    return out, res


def kernel(**inputs):
    out, _ = run(**inputs)
    return out
